# revision 1
# baseline (speedup 1.0000x reference)
"""Bass/Trainium2 kernel for nn_GPREDecoder (GlobalPointer relation-extraction loss).

Strategy: data-parallel over batch (B=8 -> 8 cores, 1 example per core).
The device computes, per example:
  - projT = W_all @ x_aug.T  (channel-major projection, bias folded in)
  - RoPE rotation for the two "ent" heads (J-matmul + cos/sin elementwise)
  - per-head S x S logits tiles on PE, exp(SCALE*logit) on ACT with fused
    per-row accumulation -> per-head sum(exp(masked logits))  (never
    materializing the S x S tensors in HBM)
  - outputs the per-head exp-sums and the final q/k tensors
The host gathers the 64 ground-truth pairs per head from q/k, applies the
multilabel-CE pos/neg log corrections in float64, and returns the scalar loss.
"""

import ml_dtypes
import numpy as np
from contextlib import ExitStack

import concourse.bass as bass
import concourse.mybir as mybir
import concourse.tile as tile
from concourse import bacc
from concourse.bass_utils import run_bass_kernel_spmd

B, S, HID, LAB = 8, 1024, 1024, 64
HD = 68
SCALE = 1.0 / HD**0.5
INF = 1.0e12
NCORES = 8
KPAD = 1152  # 9 * 128 contraction rows (1088 channels + 1 bias row + pad)
MTOT = 544   # total projection output channels
NEG_BIG = -1.0e9  # additive pre-scale mask; exp(SCALE*NEG_BIG) == 0 in fp32

# group order: q_ent0 k_ent0 q_ent1 k_ent1 q_head k_head q_tail k_tail
_GROUP_ORIG = [0, 68, 136, 204, 272, 340, 408, 476]
# heads: (q_group, k_group, tril?)
_HEADS = [(0, 1, True), (2, 3, True), (4, 5, False), (6, 7, False)]


def _spill_slots():
    """Destination (tile, row) slots for the 4 spill groups, in order."""
    slots = []
    for t in range(4):
        slots.extend((t, r) for r in range(68, 128))
    slots.extend((4, r) for r in range(32))
    return slots


def _build_perm():
    """perm[c_new] = original channel index, for the projection output layout."""
    perm = np.zeros(MTOT, np.int64)
    for g in range(4):  # rope groups aligned at row 0 of tiles 0..3
        perm[g * 128: g * 128 + 68] = np.arange(_GROUP_ORIG[g], _GROUP_ORIG[g] + 68)
    slots = _spill_slots()
    pos = 0
    for g in range(4, 8):
        for j in range(68):
            t, r = slots[pos]
            perm[t * 128 + r] = _GROUP_ORIG[g] + j
            pos += 1
    return perm


def _spill_pieces():
    """Per spill group: contiguous (src_tile, src_row0, cnt, dst_row0) DMA pieces."""
    slots = _spill_slots()
    out = {g: [] for g in range(4, 8)}
    pos = 0
    for g in range(4, 8):
        j = 0
        while j < 68:
            t, r = slots[pos]
            cnt = 1
            while j + cnt < 68 and pos + cnt < len(slots) and \
                    slots[pos + cnt] == (t, r + cnt):
                cnt += 1
            out[g].append((t, r, cnt, j))
            pos += cnt
            j += cnt
    return out


def _round_chunks(mtiles):
    """Chunk m-tiles of one [128,1024] psum round into bank-fitting matmul chunks.

    mtiles: [(m, local_start, width)] with local starts such that every
    <=512 chunk stays inside one 512-col bank. Returns
    [(m, local_off, src_off, n)] and the single contiguous ACT span end.
    """
    chunks = []
    for (m, lo, w) in mtiles:
        off = 0
        while off < w:
            n = min(512 - ((lo + off) % 512), w - off)
            chunks.append((m, lo + off, off, n))
            off += n
    return chunks


def _head_rounds(is_tril):
    """Per head: list of rounds; each round = (mtiles, span_end).

    Rounds target [128, 1024] (2-bank) psum tiles. For tril heads the
    m-tile widths shrink (only columns >= 128*m are live), so later
    m-tiles are packed two per round; spans stay contiguous from 0.
    """
    if not is_tril:
        return [([(m, 0, 1024)], 1024) for m in range(8)]
    widths = [1024 - 128 * m for m in range(8)]
    rounds = []
    for group in ((0,), (1,), (2, 6), (3, 7), (4, 5)):
        mtiles = []
        local = 0
        for m in group:
            mtiles.append((m, local, widths[m]))
            local += widths[m]
        rounds.append((mtiles, local))
    return rounds


def _n_act_cols(is_tril):
    return len(_head_rounds(is_tril))


_ACC_COLS = [_n_act_cols(t) for _, _, t in _HEADS]          # per head
_ACC_OFF = np.concatenate([[0], np.cumsum(_ACC_COLS)])      # col offset per head
SUMS_COLS = int(_ACC_OFF[-1])                               # total accum columns


def _build_nc():
    f32 = mybir.dt.float32
    # float32r: same 4-byte storage, but the PE streams it at full rate
    # (strict fp32 runs as 2 half-speed passes = 4x slower). The q/k logits
    # path jmat -> dense -> qk is typed f32r end-to-end. The projection
    # inputs are bf16 to halve the HBM load volume.
    f32r = mybir.dt.float32r
    bf16 = mybir.dt.bfloat16
    Exp = mybir.ActivationFunctionType.Exp

    nc = bacc.Bacc("TRN2", target_bir_lowering=False)

    xT = nc.dram_tensor("xT", [KPAD, S], bf16, kind="ExternalInput")
    wtb = nc.dram_tensor("wtb", [KPAD, MTOT], bf16, kind="ExternalInput")
    trig = nc.dram_tensor("trig", [HD, 2 * S], f32, kind="ExternalInput")
    jtril = nc.dram_tensor("jtril", [128, 256], f32r, kind="ExternalInput")
    sums = nc.dram_tensor("sums", [128, SUMS_COLS], f32, kind="ExternalOutput")
    qkout = nc.dram_tensor("qkout", [8, HD, S], f32r, kind="ExternalOutput")

    xT_r = xT.rearrange("(o p) f -> p o f", p=128)    # [128, 9, 1024]
    wtb_r = wtb.rearrange("(o p) f -> p o f", p=128)  # [128, 9, 544]
    KT_CHUNKS = [(0, 2), (2, 3), (5, 4)]              # (kt0, n_kt) DMA chunks

    with tile.TileContext(nc) as tc, ExitStack() as ctx:
        singles = ctx.enter_context(tc.tile_pool(name="singles", bufs=1))
        scratch = ctx.enter_context(tc.tile_pool(name="scratch", bufs=2))

        xT_sb = singles.tile([128, 9, S], bf16, tag="xT_sb", name="xT_sb")
        wtb_sb = singles.tile([128, 9, MTOT], bf16, tag="wtb_sb", name="wtb_sb")
        trig_sb = singles.tile([HD, 2 * S], f32, tag="trig_sb", name="trig_sb")
        jtril_sb = singles.tile([128, 256], f32r, tag="jtril_sb", name="jtril_sb")
        dense = [singles.tile([128, S], f32r, tag=f"dense{t}", name=f"dense{t}")
                 for t in range(5)]
        qk = [singles.tile([HD, S], f32r, tag=f"qk{g}", name=f"qk{g}")
              for g in range(8)]
        sums_sb = singles.tile([128, SUMS_COLS], f32, tag="sums_sb", name="sums_sb")
        dummy = singles.tile([1, 8], f32, tag="dummy", name="dummy")

        cos_sb = trig_sb[:, 0:S]
        sin_sb = trig_sb[:, S:2 * S]
        jmat_sb = jtril_sb[:, 0:128]
        tril_sb = jtril_sb[:, 128:256].bitcast(f32)

        # Early: zero accumulators; pre-warm the ACT exp table load.
        nc.vector.memset(sums_sb[:], 0.0)
        nc.vector.memset(dummy[:], 0.0)
        nc.scalar.activation(dummy[:], dummy[:], Exp)

        # input DMAs: first kt chunk first so the projection starts ASAP;
        # constants (needed only ~10us in) after the first chunk.
        def in_chunk(ci):
            kt0, nkt = KT_CHUNKS[ci]
            nc.sync.dma_start(out=wtb_sb[:, kt0:kt0 + nkt],
                              in_=wtb_r[:, kt0:kt0 + nkt])
            nc.scalar.dma_start(out=xT_sb[:, kt0:kt0 + nkt],
                                in_=xT_r[:, kt0:kt0 + nkt])

        in_chunk(0)
        nc.sync.dma_start(out=jtril_sb[:], in_=jtril[:, :])
        nc.scalar.dma_start(out=trig_sb[:], in_=trig[:, :])
        in_chunk(1)
        in_chunk(2)

        ps = ctx.enter_context(tc.tile_pool(name="ps", bufs=4, space="PSUM"))

        def proj_tile(t, pt, kt_lo=0, kt_hi=9):
            lo = t * 128
            hi = min(lo + 128, MTOT)
            for kt in range(kt_lo, kt_hi):
                for c in (0, 512):
                    nc.tensor.matmul(
                        pt[0:hi - lo, c:c + 512],
                        wtb_sb[:, kt, lo:hi],
                        xT_sb[:, kt, c:c + 512],
                        start=(kt == 0), stop=(kt == 8),
                    )

        def evac(t, pt, eng):
            hi = min(128, MTOT - t * 128)
            if eng == "act":
                nc.scalar.copy(out=dense[t][0:hi, :], in_=pt[0:hi, :])
            else:
                nc.vector.tensor_copy(out=dense[t][0:hi, :], in_=pt[0:hi, :])

        def jrot(g):
            """J-matmul for rope group g; returns the psum tile to release."""
            pj = ps.tile([128, S], f32, tag="ps", name=f"jq{g}")
            for c in (0, 512):
                nc.tensor.matmul(pj[:, c:c + 512], jmat_sb,
                                 dense[g][:, c:c + 512], start=True, stop=True)
            return pj

        def rope(g, pj):
            # qk[g] = dense[g]*cos + (J @ dense[g])*sin
            nc.gpsimd.tensor_tensor(qk[g][:, :], dense[g][0:HD, :], cos_sb,
                                    mybir.AluOpType.mult)
            rtmp = scratch.tile([HD, S], f32, tag="rtmp", name=f"rtmp{g}")
            nc.vector.tensor_tensor(rtmp[:, :], pj[0:HD, :], sin_sb,
                                    mybir.AluOpType.mult)
            nc.vector.tensor_tensor(qk[g][:, :], qk[g][:, :], rtmp[:, :],
                                    mybir.AluOpType.add)

        def head_logits(h, interleave=None):
            gq, gk, is_tril = _HEADS[h]
            acc = int(_ACC_OFF[h])
            for ri, (mtiles, span_end) in enumerate(_head_rounds(is_tril)):
                pl = ps.tile([128, S], f32, tag="ps", name=f"l{h}_{ri}")
                for (m, lo, so, n) in _round_chunks(mtiles):
                    g0 = 128 * m if is_tril else 0
                    nc.tensor.matmul(
                        pl[:, lo:lo + n],
                        qk[gq][:, m * 128:(m + 1) * 128],
                        qk[gk][:, g0 + so:g0 + so + n],
                        start=True, stop=True,
                    )
                if is_tril:
                    for (m, lo, w) in mtiles:
                        nc.vector.tensor_tensor(
                            pl[:, lo:lo + 128], pl[:, lo:lo + 128],
                            tril_sb, mybir.AluOpType.add)
                nc.scalar.activation(
                    pl[:, 0:span_end], pl[:, 0:span_end], Exp, scale=SCALE,
                    accum_out=sums_sb[:, acc:acc + 1])
                acc += 1
                if interleave is not None:
                    interleave(ri)
            assert acc == int(_ACC_OFF[h + 1])

        # ---- phase B1: projection tiles 0,1 (the ent-h0 rope groups) ----
        pt0 = ps.tile([128, S], f32, tag="ps", name="proj0")
        pt1 = ps.tile([128, S], f32, tag="ps", name="proj1")
        for kt in range(9):
            for t, pt in ((0, pt0), (1, pt1)):
                for c in (0, 512):
                    nc.tensor.matmul(pt[:, c:c + 512],
                                     wtb_sb[:, kt, t * 128:(t + 1) * 128],
                                     xT_sb[:, kt, c:c + 512],
                                     start=(kt == 0), stop=(kt == 8))
        evac(0, pt0, "dve")
        evac(1, pt1, "dve")
        pj0 = jrot(0)
        pj1 = jrot(1)
        rope(0, pj0)
        rope(1, pj1)
        nc.sync.dma_start(out=qkout[0], in_=qk[0][:, :])
        nc.scalar.dma_start(out=qkout[1], in_=qk[1][:, :])

        # ---- ent head 0: starts the ACT exp stream as early as possible ----
        head_logits(0)

        # ---- phase B2: projection tiles 2,3 ----
        pt2 = ps.tile([128, S], f32, tag="ps", name="proj2")
        pt3 = ps.tile([128, S], f32, tag="ps", name="proj3")
        proj_tile(2, pt2)
        proj_tile(3, pt3)
        evac(2, pt2, "dve")
        evac(3, pt3, "dve")

        # ---- phase B3: projection tile 4 + spill regroup for head/tail ----
        pt4 = ps.tile([128, S], f32, tag="ps", name="proj4")
        proj_tile(4, pt4)
        evac(4, pt4, "dve")
        for g, pieces in _spill_pieces().items():
            for i, (t, r0, cnt, d0) in enumerate(pieces):
                eng = nc.sync if (g + i) % 2 == 0 else nc.scalar
                eng.dma_start(out=qk[g][d0:d0 + cnt, :],
                              in_=dense[t][r0:r0 + cnt, :])
            eng = nc.sync if g % 2 == 0 else nc.scalar
            eng.dma_start(out=qkout[g], in_=qk[g][:, :])

        # ---- rope for ent head 1 while the head/tail spill DMAs run ----
        pj2 = jrot(2)
        pj3 = jrot(3)
        rope(2, pj2)
        rope(3, pj3)
        nc.sync.dma_start(out=qkout[2], in_=qk[2][:, :])
        nc.scalar.dma_start(out=qkout[3], in_=qk[3][:, :])

        # ---- remaining heads: head first (its deps finish earliest) ----
        head_logits(2)
        head_logits(1)
        head_logits(3)

        nc.sync.dma_start(out=sums[:, :], in_=sums_sb[:, :])

    nc.finalize()
    return nc


_NC_CACHE = None


def _get_nc():
    global _NC_CACHE
    if _NC_CACHE is None:
        _NC_CACHE = _build_nc()
    return _NC_CACHE


def _host_tables():
    pos = np.arange(S, dtype=np.float64)[:, None]
    inv = np.power(10000.0, -2.0 * np.arange(HD // 2, dtype=np.float64) / HD)
    ang = pos * inv                                   # [S, 34]
    trig = np.zeros((HD, 2 * S), np.float32)
    trig[:, 0:S] = np.repeat(np.cos(ang), 2, axis=1).T
    trig[:, S:2 * S] = np.repeat(np.sin(ang), 2, axis=1).T
    jtril = np.zeros((128, 256), np.float32)          # [:, :128]=J.T, [:, 128:]=tril
    for i in range(HD // 2):
        # J[2i, 2i+1] = -1 ; J[2i+1, 2i] = +1  -> stored transposed
        jtril[2 * i + 1, 2 * i] = -1.0
        jtril[2 * i, 2 * i + 1] = 1.0
    jtril[:, 128:256] = np.where(
        np.arange(128)[None, :] >= np.arange(128)[:, None], 0.0, NEG_BIG)
    return trig, jtril


def _mcce_host(E_dev, q, k, gt):
    """pos/neg multilabel-CE for one (example, head). q,k: [68,S] f32; gt: [P,2]."""
    i = gt[:, 0].astype(np.int64)
    j = gt[:, 1].astype(np.int64)
    flat = i * S + j
    lv = np.sum(q[:, i].astype(np.float64) * k[:, j].astype(np.float64),
                axis=0) * SCALE                       # [P]
    live = flat != 0
    pos_loss = np.log1p(np.sum(np.exp(-lv[live])))
    l00 = float(np.sum(q[:, 0].astype(np.float64) * k[:, 0].astype(np.float64))
                * SCALE)
    uf, ui = np.unique(flat, return_index=True)
    keep = uf != 0
    excl = np.exp(l00) + np.sum(np.exp(lv[ui[keep]]))
    neg_loss = np.log1p(E_dev - excl)
    return pos_loss + neg_loss


def _reference_numpy(hidden, entity_labels, attention_mask, gt_entity, gt_head,
                     gt_tail, ent_emb, W_ent, b_ent, W_head, b_head, W_tail,
                     b_tail):
    """Slow exact numpy fallback (used only if attention_mask is not all-ones)."""
    x = np.concatenate([hidden, ent_emb[entity_labels]], axis=-1)

    def rope(v):
        b, s, h, d = v.shape
        pos = np.arange(s, dtype=np.float32)[:, None]
        inv = np.power(10000.0, -2.0 * np.arange(d // 2, dtype=np.float32) / d)
        ang = pos * inv
        sin = np.repeat(np.sin(ang), 2, axis=-1)[None, :, None, :]
        cos = np.repeat(np.cos(ang), 2, axis=-1)[None, :, None, :]
        v2 = np.stack([-v[..., 1::2], v[..., ::2]], axis=-1).reshape(v.shape)
        return v * cos + v2 * sin

    def gp(x, W, b, mask, heads, use_rope, tril):
        bx, sx, _ = x.shape
        proj = (x @ W.T + b).reshape(bx, sx, heads, 2 * HD)
        qw, kw = proj[..., :HD], proj[..., HD:]
        if use_rope:
            qw, kw = rope(qw), rope(kw)
        logits = np.einsum('bmhd,bnhd->bhmn', qw, kw) * SCALE
        pad = mask[:, None, None, :]
        logits = logits * pad - (1.0 - pad) * INF
        if tril:
            logits = logits - np.tril(np.ones((sx, sx), np.float32), -1) * INF
        return logits

    def mcce(y_true, y_pred):
        bx, hx, sx, _ = y_pred.shape
        flat = y_true[..., 0].astype(np.int64) * sx + y_true[..., 1]
        yp = y_pred.reshape(bx, hx, sx * sx).astype(np.float64)
        total = 0.0
        for b in range(bx):
            for h in range(hx):
                f = flat[b, h]
                live = f != 0
                lv = yp[b, h][f]
                pos = np.log1p(np.sum(np.exp(-lv[live])))
                neg_terms = yp[b, h].copy()
                neg_terms[0] = -np.inf
                neg_terms[np.unique(f)] = -np.inf
                neg = np.log1p(np.sum(np.exp(neg_terms)))
                total += pos + neg
        return total

    loss = 0.0
    loss += mcce(gt_entity, gp(x, W_ent, b_ent, attention_mask, 2, True, True))
    loss += mcce(gt_head, gp(x, W_head, b_head, attention_mask, 1, False, False))
    loss += mcce(gt_tail, gp(x, W_tail, b_tail, attention_mask, 1, False, False))
    return np.array(loss, dtype=np.float32)


def kernel(hidden, entity_labels, attention_mask, gt_entity, gt_head, gt_tail,
           ent_emb, W_ent, b_ent, W_head, b_head, W_tail, b_tail,
           _want_trace=False):
    hidden = np.asarray(hidden, np.float32)
    entity_labels = np.asarray(entity_labels)
    attention_mask = np.asarray(attention_mask, np.float32)
    ent_emb = np.asarray(ent_emb, np.float32)

    if not np.all(attention_mask == 1.0):
        return _reference_numpy(
            hidden, entity_labels, attention_mask, np.asarray(gt_entity),
            np.asarray(gt_head), np.asarray(gt_tail), ent_emb,
            np.asarray(W_ent, np.float32), np.asarray(b_ent, np.float32),
            np.asarray(W_head, np.float32), np.asarray(b_head, np.float32),
            np.asarray(W_tail, np.float32), np.asarray(b_tail, np.float32))

    W_all = np.concatenate(
        [np.asarray(W_ent, np.float32), np.asarray(W_head, np.float32),
         np.asarray(W_tail, np.float32)], axis=0)       # [544, 1088]
    b_all = np.concatenate(
        [np.asarray(b_ent, np.float32), np.asarray(b_head, np.float32),
         np.asarray(b_tail, np.float32)], axis=0)       # [544]
    perm = _build_perm()
    Wp, bp = W_all[perm], b_all[perm]
    wtb = np.zeros((KPAD, MTOT), np.float32)
    wtb[:HID + LAB] = Wp.T
    wtb[HID + LAB] = bp
    wtb = wtb.astype(ml_dtypes.bfloat16)

    trig, jtril = _host_tables()

    in_maps = []
    for b in range(B):
        xT = np.zeros((KPAD, S), np.float32)
        xT[:HID] = hidden[b].T
        xT[HID:HID + LAB] = ent_emb[entity_labels[b]].T
        xT[HID + LAB] = 1.0
        in_maps.append(dict(xT=xT.astype(ml_dtypes.bfloat16), wtb=wtb,
                            trig=trig, jtril=jtril))

    nc = _get_nc()
    res = run_bass_kernel_spmd(nc, in_maps, core_ids=list(range(NCORES)),
                               trace=_want_trace)

    gts = {0: np.asarray(gt_entity), 2: np.asarray(gt_head),
           3: np.asarray(gt_tail)}
    total = 0.0
    for b in range(B):
        out = res.results[b]
        sums = out["sums"].astype(np.float64)      # [128, SUMS_COLS]
        qkv = out["qkout"]                         # [8, 68, 1024]
        for h, (gq, gk, is_tril) in enumerate(_HEADS):
            E = float(np.sum(sums[:, _ACC_OFF[h]:_ACC_OFF[h + 1]]))
            if h < 2:
                gt = gts[0][b, h]
            else:
                gt = gts[h][b, 0]
            total += _mcce_host(E, qkv[gq], qkv[gk], gt)

    if _want_trace:
        kernel._last_results = res
    return np.array(total, dtype=np.float32)



# revision 3
# speedup vs baseline: 1.3209x; 1.3209x over previous
"""Bass/Trainium2 kernel for nn_GPREDecoder (GlobalPointer relation-extraction loss).

Strategy: data-parallel over batch (B=8 -> 8 cores, 1 example per core).
Per example on-device:
  - projT = W_all @ x_aug.T via fp8-e4m3 DoubleRow matmuls (2 k-tiles per
    PE pass; weights pre-scaled x16 for fp8 range) plus a small bf16
    one-hot matmul for the label-embedding + bias contribution
    (table = W_emb @ ent_emb.T + b precomputed exactly on host).
  - RoPE rotation for the two "ent" heads (J-matmul + cos/sin elementwise,
    trig tables pre-divided by 16 so the ent q/k come out exactly scaled)
  - per-head S x S logits tiles on PE (bf16), exp(scale*logit) on ACT with
    fused per-row accumulation -> per-head sum(exp(masked logits)); the
    S x S tensors never touch HBM.
  - outputs per-head exp-sums and the final q/k tensors (bf16).
The host gathers the 64 ground-truth pairs per head from q/k, applies the
multilabel-CE pos/neg log corrections in float64, and returns the scalar loss.
"""

import ml_dtypes
import numpy as np
from contextlib import ExitStack

import concourse.bass as bass
import concourse.mybir as mybir
import concourse.tile as tile
from concourse import bacc
from concourse.bass_utils import run_bass_kernel_spmd

B, S, HID, LAB = 8, 1024, 1024, 64
HD = 68
SCALE = 1.0 / HD**0.5
INF = 1.0e12
NCORES = 8
MTOT = 544   # total projection output channels
WSCALE = 16.0  # fp8 weight pre-scale (keeps 0.03-sigma weights in e4m3 normals)
NEG_BIG = -1.0e9  # additive pre-scale mask; exp(SCALE*NEG_BIG) == 0 in fp32

FP8 = ml_dtypes.float8_e4m3
BF16 = ml_dtypes.bfloat16

# group order: q_ent0 k_ent0 q_ent1 k_ent1 q_head k_head q_tail k_tail
_GROUP_ORIG = [0, 68, 136, 204, 272, 340, 408, 476]
# heads: (q_group, k_group, tril?)  heads 0,1 = ent (rope, exact scale);
# heads 2,3 = head/tail (q/k carry the x16 factor -> exp scale /256)
_HEADS = [(0, 1, True), (2, 3, True), (4, 5, False), (6, 7, False)]


def _spill_slots():
    """Destination (tile, row) slots for the 4 spill groups, in order."""
    slots = []
    for t in range(4):
        slots.extend((t, r) for r in range(68, 128))
    slots.extend((4, r) for r in range(32))
    return slots


def _build_perm():
    """perm[c_new] = original channel index, for the projection output layout."""
    perm = np.zeros(MTOT, np.int64)
    for g in range(4):  # rope groups aligned at row 0 of tiles 0..3
        perm[g * 128: g * 128 + 68] = np.arange(_GROUP_ORIG[g], _GROUP_ORIG[g] + 68)
    slots = _spill_slots()
    pos = 0
    for g in range(4, 8):
        for j in range(68):
            t, r = slots[pos]
            perm[t * 128 + r] = _GROUP_ORIG[g] + j
            pos += 1
    return perm


def _spill_pieces():
    """Per spill group: contiguous (src_tile, src_row0, cnt, dst_row0) DMA pieces."""
    slots = _spill_slots()
    out = {g: [] for g in range(4, 8)}
    pos = 0
    for g in range(4, 8):
        j = 0
        while j < 68:
            t, r = slots[pos]
            cnt = 1
            while j + cnt < 68 and pos + cnt < len(slots) and \
                    slots[pos + cnt] == (t, r + cnt):
                cnt += 1
            out[g].append((t, r, cnt, j))
            pos += cnt
            j += cnt
    return out


def _round_chunks(mtiles):
    """Chunk m-tiles of one [128,1024] psum round into bank-fitting matmul chunks.

    mtiles: [(m, local_start, width)] with local starts such that every
    <=512 chunk stays inside one 512-col bank. Returns
    [(m, local_off, src_off, n)].
    """
    chunks = []
    for (m, lo, w) in mtiles:
        off = 0
        while off < w:
            n = min(512 - ((lo + off) % 512), w - off)
            chunks.append((m, lo + off, off, n))
            off += n
    return chunks


def _head_rounds(is_tril):
    """Per head: list of rounds; each round = (mtiles, span_end).

    Rounds target [128, 1024] (2-bank) psum tiles. For tril heads the
    m-tile widths shrink (only columns >= 128*m are live), so later
    m-tiles are packed two per round; spans stay contiguous from 0.
    """
    if not is_tril:
        return [([(m, 0, 1024)], 1024) for m in range(8)]
    widths = [1024 - 128 * m for m in range(8)]
    rounds = []
    for group in ((0,), (1,), (2, 6), (3, 7), (4, 5)):
        mtiles = []
        local = 0
        for m in group:
            mtiles.append((m, local, widths[m]))
            local += widths[m]
        rounds.append((mtiles, local))
    return rounds


def _n_act_cols(is_tril):
    return len(_head_rounds(is_tril))


_ACC_COLS = [_n_act_cols(t) for _, _, t in _HEADS]          # per head
_ACC_OFF = np.concatenate([[0], np.cumsum(_ACC_COLS)])      # col offset per head
SUMS_COLS = int(_ACC_OFF[-1])                               # total accum columns


def _build_nc():
    f32 = mybir.dt.float32
    bf16 = mybir.dt.bfloat16
    fp8 = mybir.dt.float8e4
    Exp = mybir.ActivationFunctionType.Exp
    DR = mybir.MatmulPerfMode.DoubleRow

    nc = bacc.Bacc("TRN2", target_bir_lowering=False)

    xT = nc.dram_tensor("xT", [128, 8 * S], fp8, kind="ExternalInput")
    wtb = nc.dram_tensor("wtb", [128, 8 * MTOT], fp8, kind="ExternalInput")
    oh = nc.dram_tensor("oh", [3, S], bf16, kind="ExternalInput")
    tbl = nc.dram_tensor("tbl", [3, MTOT], bf16, kind="ExternalInput")
    trig = nc.dram_tensor("trig", [HD, 2 * S], bf16, kind="ExternalInput")
    jt = nc.dram_tensor("jt", [128, 128], bf16, kind="ExternalInput")
    tril = nc.dram_tensor("tril", [128, 128], f32, kind="ExternalInput")
    sums = nc.dram_tensor("sums", [128, SUMS_COLS], f32, kind="ExternalOutput")
    qkout = nc.dram_tensor("qkout", [8, HD, S], bf16, kind="ExternalOutput")

    xT_r = xT.rearrange("p (o f) -> p o f", f=S)        # [128, 8, 1024]
    wtb_r = wtb.rearrange("p (o f) -> p o f", f=MTOT)   # [128, 8, 544]

    with tile.TileContext(nc) as tc, ExitStack() as ctx:
        singles = ctx.enter_context(tc.tile_pool(name="singles", bufs=1))
        scratch = ctx.enter_context(tc.tile_pool(name="scratch", bufs=2))

        xT_sb = singles.tile([128, 8, S], fp8, tag="xT_sb", name="xT_sb")
        wtb_sb = singles.tile([128, 8, MTOT], fp8, tag="wtb_sb", name="wtb_sb")
        oh_sb = singles.tile([3, S], bf16, tag="oh_sb", name="oh_sb")
        tbl_sb = singles.tile([3, MTOT], bf16, tag="tbl_sb", name="tbl_sb")
        trig_sb = singles.tile([HD, 2 * S], bf16, tag="trig_sb", name="trig_sb")
        jt_sb = singles.tile([128, 128], bf16, tag="jt_sb", name="jt_sb")
        tril_sb = singles.tile([128, 128], f32, tag="tril_sb", name="tril_sb")
        dense = [singles.tile([128, S], bf16, tag=f"dense{t}", name=f"dense{t}")
                 for t in range(5)]
        qk = [singles.tile([HD, S], bf16, tag=f"qk{g}", name=f"qk{g}")
              for g in range(8)]
        sums_sb = singles.tile([128, SUMS_COLS], f32, tag="sums_sb", name="sums_sb")
        dummy = singles.tile([1, 8], f32, tag="dummy", name="dummy")

        cos_sb = trig_sb[:, 0:S]
        sin_sb = trig_sb[:, S:2 * S]

        # Early: zero accumulators; pre-warm the ACT exp table load.
        nc.vector.memset(sums_sb[:], 0.0)
        nc.vector.memset(dummy[:], 0.0)
        nc.scalar.activation(dummy[:], dummy[:], Exp)

        # Input DMAs. Triggers only on sync/vector/gpsimd queues (scalar
        # runs the exp stream, tensor the matmul stream). First k-pair of
        # weights+activations first so the projection starts ASAP.
        for p in range(4):
            nc.sync.dma_start(out=wtb_sb[:, 2 * p:2 * p + 2],
                              in_=wtb_r[:, 2 * p:2 * p + 2])
            nc.gpsimd.dma_start(out=xT_sb[:, 2 * p:2 * p + 2],
                                in_=xT_r[:, 2 * p:2 * p + 2])
            if p == 0:
                nc.sync.dma_start(out=tbl_sb[:], in_=tbl[:, :])
                nc.gpsimd.dma_start(out=oh_sb[:], in_=oh[:, :])
        nc.gpsimd.dma_start(out=trig_sb[:], in_=trig[:, :])
        nc.sync.dma_start(out=jt_sb[:], in_=jt[:, :])
        nc.sync.dma_start(out=tril_sb[:], in_=tril[:, :])

        ps = ctx.enter_context(tc.tile_pool(name="ps", bufs=4, space="PSUM"))

        def proj_pair(p, t, pt):
            lo = t * 128
            hi = min(lo + 128, MTOT)
            for c in (0, 512):
                nc.tensor.matmul(
                    pt[0:hi - lo, c:c + 512],
                    wtb_sb[:, 2 * p:2 * p + 2, lo:hi],
                    xT_sb[:, 2 * p:2 * p + 2, c:c + 512],
                    start=(p == 0), stop=False, perf_mode=DR,
                )

        def proj_oh(t, pt):
            lo = t * 128
            hi = min(lo + 128, MTOT)
            for c in (0, 512):
                nc.tensor.matmul(
                    pt[0:hi - lo, c:c + 512],
                    tbl_sb[:, lo:hi],
                    oh_sb[:, c:c + 512],
                    start=False, stop=True,
                )

        def proj_tile(t, pt):
            for p in range(4):
                proj_pair(p, t, pt)
            proj_oh(t, pt)

        def evac(t, pt):
            hi = min(128, MTOT - t * 128)
            nc.vector.tensor_copy(out=dense[t][0:hi, :], in_=pt[0:hi, :])

        def jrot(g):
            """J-matmul for rope group g; returns the psum tile."""
            pj = ps.tile([128, S], f32, tag="ps", name=f"jq{g}")
            for c in (0, 512):
                nc.tensor.matmul(pj[0:HD, c:c + 512], jt_sb[:, 0:HD],
                                 dense[g][:, c:c + 512], start=True, stop=True)
            return pj

        def rope(g, pj):
            # qk[g] = dense[g]*(cos/16) + (J @ dense[g])*(sin/16)  == exact q_rot
            nc.gpsimd.tensor_tensor(qk[g][:, :], dense[g][0:HD, :], cos_sb,
                                    mybir.AluOpType.mult)
            rtmp = scratch.tile([HD, S], bf16, tag="rtmp", name=f"rtmp{g}")
            nc.vector.tensor_tensor(rtmp[:, :], pj[0:HD, :], sin_sb,
                                    mybir.AluOpType.mult)
            nc.vector.tensor_tensor(qk[g][:, :], qk[g][:, :], rtmp[:, :],
                                    mybir.AluOpType.add)

        def head_logits(h):
            gq, gk, is_tril = _HEADS[h]
            # ent q/k are exact; head/tail q/k carry x16 each -> /256
            sc = SCALE if is_tril else SCALE / (WSCALE * WSCALE)
            acc = int(_ACC_OFF[h])
            for ri, (mtiles, span_end) in enumerate(_head_rounds(is_tril)):
                pl = ps.tile([128, S], f32, tag="ps", name=f"l{h}_{ri}")
                for (m, lo, so, n) in _round_chunks(mtiles):
                    g0 = 128 * m if is_tril else 0
                    nc.tensor.matmul(
                        pl[:, lo:lo + n],
                        qk[gq][:, m * 128:(m + 1) * 128],
                        qk[gk][:, g0 + so:g0 + so + n],
                        start=True, stop=True,
                    )
                if is_tril:
                    for (m, lo, w) in mtiles:
                        nc.vector.tensor_tensor(
                            pl[:, lo:lo + 128], pl[:, lo:lo + 128],
                            tril_sb, mybir.AluOpType.add)
                nc.scalar.activation(
                    pl[:, 0:span_end], pl[:, 0:span_end], Exp, scale=sc,
                    accum_out=sums_sb[:, acc:acc + 1])
                acc += 1
            assert acc == int(_ACC_OFF[h + 1])

        # ---- phase B1: projection tiles 0,1 (the ent-h0 rope groups) ----
        pt0 = ps.tile([128, S], f32, tag="ps", name="proj0")
        pt1 = ps.tile([128, S], f32, tag="ps", name="proj1")
        for p in range(4):
            proj_pair(p, 0, pt0)
            proj_pair(p, 1, pt1)
        proj_oh(0, pt0)
        proj_oh(1, pt1)
        evac(0, pt0)
        evac(1, pt1)
        pj0 = jrot(0)
        pj1 = jrot(1)
        rope(0, pj0)
        rope(1, pj1)
        nc.sync.dma_start(out=qkout[0], in_=qk[0][:, :])
        nc.gpsimd.dma_start(out=qkout[1], in_=qk[1][:, :])

        # ---- ent head 0: starts the ACT exp stream as early as possible ----
        head_logits(0)

        # ---- phase B2: projection tiles 2,3 ----
        pt2 = ps.tile([128, S], f32, tag="ps", name="proj2")
        pt3 = ps.tile([128, S], f32, tag="ps", name="proj3")
        for p in range(4):
            proj_pair(p, 2, pt2)
            proj_pair(p, 3, pt3)
        proj_oh(2, pt2)
        proj_oh(3, pt3)
        evac(2, pt2)
        evac(3, pt3)

        # ---- phase B3: projection tile 4 + spill regroup for head/tail ----
        pt4 = ps.tile([128, S], f32, tag="ps", name="proj4")
        proj_tile(4, pt4)
        evac(4, pt4)
        engs = [nc.sync, nc.gpsimd]
        ei = 0
        for g, pieces in _spill_pieces().items():
            for (t, r0, cnt, d0) in pieces:
                engs[ei % 2].dma_start(out=qk[g][d0:d0 + cnt, :],
                                       in_=dense[t][r0:r0 + cnt, :])
                ei += 1
            engs[ei % 2].dma_start(out=qkout[g], in_=qk[g][:, :])
            ei += 1

        # ---- rope for ent head 1 while the head/tail spill DMAs run ----
        pj2 = jrot(2)
        pj3 = jrot(3)
        rope(2, pj2)
        rope(3, pj3)
        nc.sync.dma_start(out=qkout[2], in_=qk[2][:, :])
        nc.gpsimd.dma_start(out=qkout[3], in_=qk[3][:, :])

        # ---- remaining heads: head first (its deps finish earliest) ----
        head_logits(2)
        head_logits(1)
        head_logits(3)

        nc.sync.dma_start(out=sums[:, :], in_=sums_sb[:, :])

    nc.finalize()
    return nc


_NC_CACHE = None


def _get_nc():
    global _NC_CACHE
    if _NC_CACHE is None:
        _NC_CACHE = _build_nc()
    return _NC_CACHE


def _host_tables():
    pos = np.arange(S, dtype=np.float64)[:, None]
    inv = np.power(10000.0, -2.0 * np.arange(HD // 2, dtype=np.float64) / HD)
    ang = pos * inv                                   # [S, 34]
    trig = np.zeros((HD, 2 * S), np.float64)
    trig[:, 0:S] = np.repeat(np.cos(ang), 2, axis=1).T / WSCALE
    trig[:, S:2 * S] = np.repeat(np.sin(ang), 2, axis=1).T / WSCALE
    jt = np.zeros((128, 128), np.float32)
    for i in range(HD // 2):
        # J[2i, 2i+1] = -1 ; J[2i+1, 2i] = +1  -> stored transposed
        jt[2 * i + 1, 2 * i] = -1.0
        jt[2 * i, 2 * i + 1] = 1.0
    tril = np.where(np.arange(128)[None, :] >= np.arange(128)[:, None],
                    0.0, NEG_BIG).astype(np.float32)
    return trig.astype(BF16), jt.astype(BF16), tril


def _mcce_host(E_dev, q, k, gt):
    """pos/neg multilabel-CE for one (example, head). q,k: [68,S] f64; gt: [P,2]."""
    i = gt[:, 0].astype(np.int64)
    j = gt[:, 1].astype(np.int64)
    flat = i * S + j
    lv = np.sum(q[:, i] * k[:, j], axis=0) * SCALE    # [P]
    live = flat != 0
    pos_loss = np.log1p(np.sum(np.exp(-lv[live])))
    l00 = float(np.sum(q[:, 0] * k[:, 0]) * SCALE)
    uf, ui = np.unique(flat, return_index=True)
    keep = uf != 0
    excl = np.exp(l00) + np.sum(np.exp(lv[ui[keep]]))
    neg_loss = np.log1p(E_dev - excl)
    return pos_loss + neg_loss


def _reference_numpy(hidden, entity_labels, attention_mask, gt_entity, gt_head,
                     gt_tail, ent_emb, W_ent, b_ent, W_head, b_head, W_tail,
                     b_tail):
    """Slow exact numpy fallback (used only if attention_mask is not all-ones)."""
    x = np.concatenate([hidden, ent_emb[entity_labels]], axis=-1)

    def rope(v):
        b, s, h, d = v.shape
        pos = np.arange(s, dtype=np.float32)[:, None]
        inv = np.power(10000.0, -2.0 * np.arange(d // 2, dtype=np.float32) / d)
        ang = pos * inv
        sin = np.repeat(np.sin(ang), 2, axis=-1)[None, :, None, :]
        cos = np.repeat(np.cos(ang), 2, axis=-1)[None, :, None, :]
        v2 = np.stack([-v[..., 1::2], v[..., ::2]], axis=-1).reshape(v.shape)
        return v * cos + v2 * sin

    def gp(x, W, b, mask, heads, use_rope, tril):
        bx, sx, _ = x.shape
        proj = (x @ W.T + b).reshape(bx, sx, heads, 2 * HD)
        qw, kw = proj[..., :HD], proj[..., HD:]
        if use_rope:
            qw, kw = rope(qw), rope(kw)
        logits = np.einsum('bmhd,bnhd->bhmn', qw, kw) * SCALE
        pad = mask[:, None, None, :]
        logits = logits * pad - (1.0 - pad) * INF
        if tril:
            logits = logits - np.tril(np.ones((sx, sx), np.float32), -1) * INF
        return logits

    def mcce(y_true, y_pred):
        bx, hx, sx, _ = y_pred.shape
        flat = y_true[..., 0].astype(np.int64) * sx + y_true[..., 1]
        yp = y_pred.reshape(bx, hx, sx * sx).astype(np.float64)
        total = 0.0
        for b in range(bx):
            for h in range(hx):
                f = flat[b, h]
                live = f != 0
                lv = yp[b, h][f]
                pos = np.log1p(np.sum(np.exp(-lv[live])))
                neg_terms = yp[b, h].copy()
                neg_terms[0] = -np.inf
                neg_terms[np.unique(f)] = -np.inf
                neg = np.log1p(np.sum(np.exp(neg_terms)))
                total += pos + neg
        return total

    loss = 0.0
    loss += mcce(gt_entity, gp(x, W_ent, b_ent, attention_mask, 2, True, True))
    loss += mcce(gt_head, gp(x, W_head, b_head, attention_mask, 1, False, False))
    loss += mcce(gt_tail, gp(x, W_tail, b_tail, attention_mask, 1, False, False))
    return np.array(loss, dtype=np.float32)


def kernel(hidden, entity_labels, attention_mask, gt_entity, gt_head, gt_tail,
           ent_emb, W_ent, b_ent, W_head, b_head, W_tail, b_tail,
           _want_trace=False):
    hidden = np.asarray(hidden, np.float32)
    entity_labels = np.asarray(entity_labels)
    attention_mask = np.asarray(attention_mask, np.float32)
    ent_emb = np.asarray(ent_emb, np.float32)

    if not np.all(attention_mask == 1.0):
        return _reference_numpy(
            hidden, entity_labels, attention_mask, np.asarray(gt_entity),
            np.asarray(gt_head), np.asarray(gt_tail), ent_emb,
            np.asarray(W_ent, np.float32), np.asarray(b_ent, np.float32),
            np.asarray(W_head, np.float32), np.asarray(b_head, np.float32),
            np.asarray(W_tail, np.float32), np.asarray(b_tail, np.float32))

    W_all = np.concatenate(
        [np.asarray(W_ent, np.float32), np.asarray(W_head, np.float32),
         np.asarray(W_tail, np.float32)], axis=0)       # [544, 1088]
    b_all = np.concatenate(
        [np.asarray(b_ent, np.float32), np.asarray(b_head, np.float32),
         np.asarray(b_tail, np.float32)], axis=0)       # [544]
    perm = _build_perm()
    Wp, bp = W_all[perm], b_all[perm]

    # fp8 DoubleRow weights: [128, 8 (k-subtile), 544], x16 pre-scale
    wtb = np.ascontiguousarray(
        (WSCALE * Wp[:, :HID].T).reshape(8, 128, MTOT).transpose(1, 0, 2)
    ).astype(FP8).reshape(128, 8 * MTOT)
    # one-hot table: W_emb @ emb.T + bias, x16 to match the weight scale
    tbl = (WSCALE * (np.asarray(ent_emb, np.float64) @ Wp[:, HID:].T.astype(np.float64)
                     + bp[None, :].astype(np.float64))).astype(BF16)  # [3, 544]

    trig, jt, tril = _host_tables()

    in_maps = []
    for b in range(B):
        xT = np.ascontiguousarray(
            hidden[b].T.reshape(8, 128, S).transpose(1, 0, 2)
        ).astype(FP8).reshape(128, 8 * S)
        oh = (entity_labels[b][None, :] == np.arange(3)[:, None]).astype(BF16)
        in_maps.append(dict(xT=xT, oh=oh, wtb=wtb, tbl=tbl,
                            trig=trig, jt=jt, tril=tril))

    nc = _get_nc()
    res = run_bass_kernel_spmd(nc, in_maps, core_ids=list(range(NCORES)),
                               trace=_want_trace)

    gts = {0: np.asarray(gt_entity), 2: np.asarray(gt_head),
           3: np.asarray(gt_tail)}
    total = 0.0
    for b in range(B):
        out = res.results[b]
        sums = out["sums"].astype(np.float64)      # [128, SUMS_COLS]
        qkv = out["qkout"].astype(np.float64)      # [8, 68, 1024]
        qkv[4:] /= WSCALE                          # head/tail groups carry x16
        for h, (gq, gk, is_tril) in enumerate(_HEADS):
            E = float(np.sum(sums[:, _ACC_OFF[h]:_ACC_OFF[h + 1]]))
            if h < 2:
                gt = gts[0][b, h]
            else:
                gt = gts[h][b, 0]
            total += _mcce_host(E, qkv[gq], qkv[gk], gt)

    if _want_trace:
        kernel._last_results = res
    return np.array(total, dtype=np.float32)


# revision 7
# speedup vs baseline: 1.4144x; 1.0708x over previous
"""Bass/Trainium2 kernel for nn_GPREDecoder (GlobalPointer relation-extraction loss).

Strategy: data-parallel over batch (B=8 -> 8 cores, 1 example per core).
Per example on-device:
  - projT = W_all @ x_aug.T via fp8-e4m3 DoubleRow matmuls (2 k-tiles per
    PE pass; weights pre-scaled x16 for fp8 range) plus a small bf16
    one-hot matmul for the label-embedding + bias contribution
    (table = W_emb @ ent_emb.T + b precomputed exactly on host).
  - RoPE rotation for the two "ent" heads (J-matmul + cos/sin elementwise,
    trig tables pre-divided by 16 so the ent q/k come out exactly scaled)
  - per-head S x S logits tiles on PE (bf16), exp(scale*logit) on ACT with
    fused per-row accumulation -> per-head sum(exp(masked logits)); the
    S x S tensors never touch HBM.
  - outputs per-head exp-sums and the final q/k tensors (bf16).
The host gathers the 64 ground-truth pairs per head from q/k, applies the
multilabel-CE pos/neg log corrections in float64, and returns the scalar loss.
"""

import ml_dtypes
import numpy as np
from contextlib import ExitStack

import concourse.bass as bass
import concourse.mybir as mybir
import concourse.tile as tile
from concourse import bacc
from concourse.bass_utils import run_bass_kernel_spmd

B, S, HID, LAB = 8, 1024, 1024, 64
HD = 68
SCALE = 1.0 / HD**0.5
INF = 1.0e12
NCORES = 8
MTOT = 544   # total projection output channels
WSCALE = 16.0  # fp8 weight pre-scale (keeps 0.03-sigma weights in e4m3 normals)
NEG_BIG = -1.0e9  # additive pre-scale mask; exp(SCALE*NEG_BIG) == 0 in fp32

FP8 = ml_dtypes.float8_e4m3
BF16 = ml_dtypes.bfloat16

# group order: q_ent0 k_ent0 q_ent1 k_ent1 q_head k_head q_tail k_tail
_GROUP_ORIG = [0, 68, 136, 204, 272, 340, 408, 476]
# heads: (q_group, k_group, tril?)  heads 0,1 = ent (rope, exact scale);
# heads 2,3 = head/tail (q/k carry the x16 factor -> exp scale /256)
_HEADS = [(0, 1, True), (2, 3, True), (4, 5, False), (6, 7, False)]


def _spill_slots():
    """Destination (tile, row) slots for the 4 spill groups, in order."""
    slots = []
    for t in range(4):
        slots.extend((t, r) for r in range(68, 128))
    slots.extend((4, r) for r in range(32))
    return slots


def _build_perm():
    """perm[c_new] = original channel index, for the projection output layout."""
    perm = np.zeros(MTOT, np.int64)
    for g in range(4):  # rope groups aligned at row 0 of tiles 0..3
        perm[g * 128: g * 128 + 68] = np.arange(_GROUP_ORIG[g], _GROUP_ORIG[g] + 68)
    slots = _spill_slots()
    pos = 0
    for g in range(4, 8):
        for j in range(68):
            t, r = slots[pos]
            perm[t * 128 + r] = _GROUP_ORIG[g] + j
            pos += 1
    return perm


def _spill_pieces():
    """Per spill group: contiguous (src_tile, src_row0, cnt, dst_row0) DMA pieces."""
    slots = _spill_slots()
    out = {g: [] for g in range(4, 8)}
    pos = 0
    for g in range(4, 8):
        j = 0
        while j < 68:
            t, r = slots[pos]
            cnt = 1
            while j + cnt < 68 and pos + cnt < len(slots) and \
                    slots[pos + cnt] == (t, r + cnt):
                cnt += 1
            out[g].append((t, r, cnt, j))
            pos += cnt
            j += cnt
    return out


def _round_chunks(mtiles):
    """Chunk m-tiles of one [128,1024] psum round into bank-fitting matmul chunks.

    mtiles: [(m, local_start, width)] with local starts such that every
    <=512 chunk stays inside one 512-col bank. Returns
    [(m, local_off, src_off, n)].
    """
    chunks = []
    for (m, lo, w) in mtiles:
        off = 0
        while off < w:
            n = min(512 - ((lo + off) % 512), w - off)
            chunks.append((m, lo + off, off, n))
            off += n
    return chunks


def _head_rounds(is_tril, wide):
    """Per head: list of rounds; each round = (mtiles, span_end).

    Narrow rounds target [128, 1024] (2-bank) psum tiles, wide rounds
    [128, 2048] (4-bank). For tril heads the m-tile widths shrink (only
    columns >= 128*m are live), so m-tiles pack; spans stay contiguous.
    """
    if not is_tril:
        if not wide:
            return [([(m, 0, 1024)], 1024) for m in range(8)]
        return [([(2 * r, 0, 1024), (2 * r + 1, 1024, 1024)], 2048)
                for r in range(4)]
    widths = [1024 - 128 * m for m in range(8)]
    groups = ((0,), (1,), (2, 6), (3, 7), (4, 5)) if not wide else         ((0, 1), (2, 3, 6), (4, 5, 7))
    rounds = []
    for group in groups:
        mtiles = []
        local = 0
        for m in group:
            mtiles.append((m, local, widths[m]))
            local += widths[m]
        rounds.append((mtiles, local))
    return rounds


# all heads currently narrow (wide pool swap crashed the exec unit)
_HEAD_WIDE = [False, False, False, False]
_ACC_COLS = [len(_head_rounds(t, _HEAD_WIDE[h]))
             for h, (_, _, t) in enumerate(_HEADS)]         # [5, 3, 4, 4]
_ACC_OFF = np.concatenate([[0], np.cumsum(_ACC_COLS)])      # col offset per head
SUMS_COLS = int(_ACC_OFF[-1])                               # total accum columns


def _build_nc():
    f32 = mybir.dt.float32
    bf16 = mybir.dt.bfloat16
    fp8 = mybir.dt.float8e4
    Exp = mybir.ActivationFunctionType.Exp
    DR = mybir.MatmulPerfMode.DoubleRow

    nc = bacc.Bacc("TRN2", target_bir_lowering=False)

    xT = nc.dram_tensor("xT", [128, 8 * S], fp8, kind="ExternalInput")
    wtb = nc.dram_tensor("wtb", [128, 8 * MTOT], fp8, kind="ExternalInput")
    oh = nc.dram_tensor("oh", [3, S], bf16, kind="ExternalInput")
    tbl = nc.dram_tensor("tbl", [3, MTOT], bf16, kind="ExternalInput")
    trig = nc.dram_tensor("trig", [HD, 2 * S], bf16, kind="ExternalInput")
    jt = nc.dram_tensor("jt", [128, 128], bf16, kind="ExternalInput")
    tril = nc.dram_tensor("tril", [128, 128], f32, kind="ExternalInput")
    sums = nc.dram_tensor("sums", [128, SUMS_COLS], f32, kind="ExternalOutput")
    qkout = nc.dram_tensor("qkout", [8, HD, S], bf16, kind="ExternalOutput")

    xT_r = xT.rearrange("p (o f) -> p o f", f=S)        # [128, 8, 1024]
    wtb_r = wtb.rearrange("p (o f) -> p o f", f=MTOT)   # [128, 8, 544]

    with tile.TileContext(nc) as tc, ExitStack() as ctx:
        singles = ctx.enter_context(tc.tile_pool(name="singles", bufs=1))
        scratch = ctx.enter_context(tc.tile_pool(name="scratch", bufs=2))

        xT_sb = singles.tile([128, 8, S], fp8, tag="xT_sb", name="xT_sb")
        wtb_sb = singles.tile([128, 8, MTOT], fp8, tag="wtb_sb", name="wtb_sb")
        oh_sb = singles.tile([3, S], bf16, tag="oh_sb", name="oh_sb")
        tbl_sb = singles.tile([3, MTOT], bf16, tag="tbl_sb", name="tbl_sb")
        trig_sb = singles.tile([HD, 2 * S], bf16, tag="trig_sb", name="trig_sb")
        jt_sb = singles.tile([128, 128], bf16, tag="jt_sb", name="jt_sb")
        tril_sb = singles.tile([128, 128], f32, tag="tril_sb", name="tril_sb")
        dense = [singles.tile([128, S], bf16, tag=f"dense{t}", name=f"dense{t}")
                 for t in range(5)]
        qk = [singles.tile([HD, S], bf16, tag=f"qk{g}", name=f"qk{g}")
              for g in range(8)]
        sums_sb = singles.tile([128, SUMS_COLS], f32, tag="sums_sb", name="sums_sb")
        dummy = singles.tile([1, 8], f32, tag="dummy", name="dummy")

        cos_sb = trig_sb[:, 0:S]
        sin_sb = trig_sb[:, S:2 * S]

        # Early: zero accumulators; pre-warm the ACT exp table load.
        nc.vector.memset(sums_sb[:], 0.0)
        nc.vector.memset(dummy[:], 0.0)
        nc.scalar.activation(dummy[:], dummy[:], Exp)

        # Input DMAs. Triggers only on sync/vector/gpsimd queues (scalar
        # runs the exp stream, tensor the matmul stream). First k-pair of
        # weights+activations first so the projection starts ASAP.
        for p in range(4):
            nc.sync.dma_start(out=wtb_sb[:, 2 * p:2 * p + 2],
                              in_=wtb_r[:, 2 * p:2 * p + 2])
            nc.gpsimd.dma_start(out=xT_sb[:, 2 * p:2 * p + 2],
                                in_=xT_r[:, 2 * p:2 * p + 2])
            if p == 0:
                nc.sync.dma_start(out=tbl_sb[:], in_=tbl[:, :])
                nc.gpsimd.dma_start(out=oh_sb[:], in_=oh[:, :])
        nc.gpsimd.dma_start(out=trig_sb[:], in_=trig[:, :])
        nc.sync.dma_start(out=jt_sb[:], in_=jt[:, :])
        nc.sync.dma_start(out=tril_sb[:], in_=tril[:, :])

        # PSUM: proj ring (2x 2-bank) + head0/J ring (2x 2-bank) = 8 banks.
        ps = tc.alloc_tile_pool(name="ps", bufs=2, space="PSUM")
        hj = tc.alloc_tile_pool(name="hj", bufs=2, space="PSUM")

        def proj_pair(p, t, pt):
            lo = t * 128
            hi = min(lo + 128, MTOT)
            for c in (0, 512):
                nc.tensor.matmul(
                    pt[0:hi - lo, c:c + 512],
                    wtb_sb[:, 2 * p:2 * p + 2, lo:hi],
                    xT_sb[:, 2 * p:2 * p + 2, c:c + 512],
                    start=(p == 0), stop=False, perf_mode=DR,
                )

        def proj_oh(t, pt):
            lo = t * 128
            hi = min(lo + 128, MTOT)
            for c in (0, 512):
                nc.tensor.matmul(
                    pt[0:hi - lo, c:c + 512],
                    tbl_sb[:, lo:hi],
                    oh_sb[:, c:c + 512],
                    start=False, stop=True,
                )

        # spill pieces grouped by source tile; qkout[g] once g completes
        by_tile = {t: [] for t in range(5)}
        for g, pieces in _spill_pieces().items():
            for (t, r0, cnt, d0) in pieces:
                by_tile[t].append((g, r0, cnt, d0))
        g_left = {g: len(p) for g, p in _spill_pieces().items()}
        spill_eng = [nc.sync, nc.gpsimd]

        def spill_for_tile(t):
            for i, (g, r0, cnt, d0) in enumerate(by_tile[t]):
                spill_eng[(t + i) % 2].dma_start(
                    out=qk[g][d0:d0 + cnt, :], in_=dense[t][r0:r0 + cnt, :])
                g_left[g] -= 1
                if g_left[g] == 0:
                    spill_eng[(t + i) % 2].dma_start(
                        out=qkout[g], in_=qk[g][:, :])

        def evac_act(t, pt):
            hi = min(128, MTOT - t * 128)
            nc.scalar.copy(out=dense[t][0:hi, :], in_=pt[0:hi, :])
            spill_for_tile(t)

        def evac_dve(t, pt):
            hi = min(128, MTOT - t * 128)
            nc.vector.tensor_copy(out=dense[t][0:hi, :], in_=pt[0:hi, :])
            spill_for_tile(t)

        def jrot(g):
            pj = hj.tile([128, S], f32, tag="hj", name=f"jq{g}")
            for c in (0, 512):
                nc.tensor.matmul(pj[0:HD, c:c + 512], jt_sb[:, 0:HD],
                                 dense[g][:, c:c + 512], start=True, stop=True)
            return pj

        def rope(g, pj):
            # qk[g] = dense[g]*(cos/16) + (J @ dense[g])*(sin/16) == exact q_rot
            nc.gpsimd.tensor_tensor(qk[g][:, :], dense[g][0:HD, :], cos_sb,
                                    mybir.AluOpType.mult)
            rtmp = scratch.tile([HD, S], bf16, tag="rtmp", name=f"rtmp{g}")
            nc.vector.tensor_tensor(rtmp[:, :], pj[0:HD, :], sin_sb,
                                    mybir.AluOpType.mult)
            nc.vector.tensor_tensor(qk[g][:, :], qk[g][:, :], rtmp[:, :],
                                    mybir.AluOpType.add)

        def head_round(h, ri, pool, tag, width):
            gq, gk, is_tril = _HEADS[h]
            sc = SCALE if h < 2 else SCALE / (WSCALE * WSCALE)
            mtiles, span = _head_rounds(is_tril, wide=(width == 2048))[ri]
            pl = pool.tile([128, width], f32, tag=tag, name=f"l{h}_{ri}")
            for (m, lo, so, n) in _round_chunks(mtiles):
                g0 = 128 * m if is_tril else 0
                nc.tensor.matmul(
                    pl[:, lo:lo + n],
                    qk[gq][:, m * 128:(m + 1) * 128],
                    qk[gk][:, g0 + so:g0 + so + n],
                    start=True, stop=True,
                )
            if is_tril:
                for (m, lo, w) in mtiles:
                    nc.vector.tensor_tensor(
                        pl[:, lo:lo + 128], pl[:, lo:lo + 128],
                        tril_sb, mybir.AluOpType.add)
            acc = int(_ACC_OFF[h]) + ri
            nc.scalar.activation(
                pl[:, 0:span], pl[:, 0:span], Exp, scale=sc,
                accum_out=sums_sb[:, acc:acc + 1])

        # ---- proj tiles 0,1 (ent-h0 q/k); evac via ACT (idle till head0) ----
        pt0 = ps.tile([128, S], f32, tag="ps", name="proj0")
        pt1 = ps.tile([128, S], f32, tag="ps", name="proj1")
        for p in range(4):
            proj_pair(p, 0, pt0)
            proj_pair(p, 1, pt1)
        proj_oh(0, pt0)
        evac_act(0, pt0)
        proj_oh(1, pt1)
        evac_act(1, pt1)
        pj0 = jrot(0)
        pj1 = jrot(1)
        rope(0, pj0)
        rope(1, pj1)
        nc.sync.dma_start(out=qkout[0], in_=qk[0][:, :])
        nc.gpsimd.dma_start(out=qkout[1], in_=qk[1][:, :])

        # ---- head0 rounds interleaved with proj tiles 2,3,4 on the PE ----
        pt2 = ps.tile([128, S], f32, tag="ps", name="proj2")
        proj_pair(0, 2, pt2)
        proj_pair(1, 2, pt2)
        head_round(0, 0, hj, "hj", 1024)
        proj_pair(2, 2, pt2)
        head_round(0, 1, hj, "hj", 1024)
        proj_pair(3, 2, pt2)
        proj_oh(2, pt2)
        head_round(0, 2, hj, "hj", 1024)
        evac_dve(2, pt2)
        pt3 = ps.tile([128, S], f32, tag="ps", name="proj3")
        proj_pair(0, 3, pt3)
        proj_pair(1, 3, pt3)
        head_round(0, 3, hj, "hj", 1024)
        proj_pair(2, 3, pt3)
        proj_pair(3, 3, pt3)
        proj_oh(3, pt3)
        evac_dve(3, pt3)
        pt4 = ps.tile([128, S], f32, tag="ps", name="proj4")
        proj_pair(0, 4, pt4)
        proj_pair(1, 4, pt4)
        head_round(0, 4, hj, "hj", 1024)
        proj_pair(2, 4, pt4)
        proj_pair(3, 4, pt4)
        proj_oh(4, pt4)
        evac_dve(4, pt4)

        # ---- rope for ent head 1 while the head/tail spill DMAs run ----
        pj2 = jrot(2)
        pj3 = jrot(3)
        rope(2, pj2)
        rope(3, pj3)
        nc.sync.dma_start(out=qkout[2], in_=qk[2][:, :])
        nc.gpsimd.dma_start(out=qkout[3], in_=qk[3][:, :])

        # ---- tail heads: narrow rounds alternating the two psum rings ----
        for hh, nr in ((2, 8), (3, 8), (1, 5)):
            for ri in range(nr):
                pool, tag = ((ps, "ps"), (hj, "hj"))[ri % 2]
                head_round(hh, ri, pool, tag, 1024)
        hj.release()
        ps.release()

        nc.sync.dma_start(out=sums[:, :], in_=sums_sb[:, :])

    nc.finalize()
    return nc


_NC_CACHE = None


def _get_nc():
    global _NC_CACHE
    if _NC_CACHE is None:
        _NC_CACHE = _build_nc()
    return _NC_CACHE


def _host_tables():
    pos = np.arange(S, dtype=np.float64)[:, None]
    inv = np.power(10000.0, -2.0 * np.arange(HD // 2, dtype=np.float64) / HD)
    ang = pos * inv                                   # [S, 34]
    trig = np.zeros((HD, 2 * S), np.float64)
    trig[:, 0:S] = np.repeat(np.cos(ang), 2, axis=1).T / WSCALE
    trig[:, S:2 * S] = np.repeat(np.sin(ang), 2, axis=1).T / WSCALE
    jt = np.zeros((128, 128), np.float32)
    for i in range(HD // 2):
        # J[2i, 2i+1] = -1 ; J[2i+1, 2i] = +1  -> stored transposed
        jt[2 * i + 1, 2 * i] = -1.0
        jt[2 * i, 2 * i + 1] = 1.0
    tril = np.where(np.arange(128)[None, :] >= np.arange(128)[:, None],
                    0.0, NEG_BIG).astype(np.float32)
    return trig.astype(BF16), jt.astype(BF16), tril


def _mcce_host(E_dev, q, k, gt):
    """pos/neg multilabel-CE for one (example, head). q,k: [68,S] f64; gt: [P,2]."""
    i = gt[:, 0].astype(np.int64)
    j = gt[:, 1].astype(np.int64)
    flat = i * S + j
    lv = np.sum(q[:, i] * k[:, j], axis=0) * SCALE    # [P]
    live = flat != 0
    pos_loss = np.log1p(np.sum(np.exp(-lv[live])))
    l00 = float(np.sum(q[:, 0] * k[:, 0]) * SCALE)
    uf, ui = np.unique(flat, return_index=True)
    keep = uf != 0
    excl = np.exp(l00) + np.sum(np.exp(lv[ui[keep]]))
    neg_loss = np.log1p(E_dev - excl)
    return pos_loss + neg_loss


def _reference_numpy(hidden, entity_labels, attention_mask, gt_entity, gt_head,
                     gt_tail, ent_emb, W_ent, b_ent, W_head, b_head, W_tail,
                     b_tail):
    """Slow exact numpy fallback (used only if attention_mask is not all-ones)."""
    x = np.concatenate([hidden, ent_emb[entity_labels]], axis=-1)

    def rope(v):
        b, s, h, d = v.shape
        pos = np.arange(s, dtype=np.float32)[:, None]
        inv = np.power(10000.0, -2.0 * np.arange(d // 2, dtype=np.float32) / d)
        ang = pos * inv
        sin = np.repeat(np.sin(ang), 2, axis=-1)[None, :, None, :]
        cos = np.repeat(np.cos(ang), 2, axis=-1)[None, :, None, :]
        v2 = np.stack([-v[..., 1::2], v[..., ::2]], axis=-1).reshape(v.shape)
        return v * cos + v2 * sin

    def gp(x, W, b, mask, heads, use_rope, tril):
        bx, sx, _ = x.shape
        proj = (x @ W.T + b).reshape(bx, sx, heads, 2 * HD)
        qw, kw = proj[..., :HD], proj[..., HD:]
        if use_rope:
            qw, kw = rope(qw), rope(kw)
        logits = np.einsum('bmhd,bnhd->bhmn', qw, kw) * SCALE
        pad = mask[:, None, None, :]
        logits = logits * pad - (1.0 - pad) * INF
        if tril:
            logits = logits - np.tril(np.ones((sx, sx), np.float32), -1) * INF
        return logits

    def mcce(y_true, y_pred):
        bx, hx, sx, _ = y_pred.shape
        flat = y_true[..., 0].astype(np.int64) * sx + y_true[..., 1]
        yp = y_pred.reshape(bx, hx, sx * sx).astype(np.float64)
        total = 0.0
        for b in range(bx):
            for h in range(hx):
                f = flat[b, h]
                live = f != 0
                lv = yp[b, h][f]
                pos = np.log1p(np.sum(np.exp(-lv[live])))
                neg_terms = yp[b, h].copy()
                neg_terms[0] = -np.inf
                neg_terms[np.unique(f)] = -np.inf
                neg = np.log1p(np.sum(np.exp(neg_terms)))
                total += pos + neg
        return total

    loss = 0.0
    loss += mcce(gt_entity, gp(x, W_ent, b_ent, attention_mask, 2, True, True))
    loss += mcce(gt_head, gp(x, W_head, b_head, attention_mask, 1, False, False))
    loss += mcce(gt_tail, gp(x, W_tail, b_tail, attention_mask, 1, False, False))
    return np.array(loss, dtype=np.float32)


def kernel(hidden, entity_labels, attention_mask, gt_entity, gt_head, gt_tail,
           ent_emb, W_ent, b_ent, W_head, b_head, W_tail, b_tail,
           _want_trace=False):
    hidden = np.asarray(hidden, np.float32)
    entity_labels = np.asarray(entity_labels)
    attention_mask = np.asarray(attention_mask, np.float32)
    ent_emb = np.asarray(ent_emb, np.float32)

    if not np.all(attention_mask == 1.0):
        return _reference_numpy(
            hidden, entity_labels, attention_mask, np.asarray(gt_entity),
            np.asarray(gt_head), np.asarray(gt_tail), ent_emb,
            np.asarray(W_ent, np.float32), np.asarray(b_ent, np.float32),
            np.asarray(W_head, np.float32), np.asarray(b_head, np.float32),
            np.asarray(W_tail, np.float32), np.asarray(b_tail, np.float32))

    W_all = np.concatenate(
        [np.asarray(W_ent, np.float32), np.asarray(W_head, np.float32),
         np.asarray(W_tail, np.float32)], axis=0)       # [544, 1088]
    b_all = np.concatenate(
        [np.asarray(b_ent, np.float32), np.asarray(b_head, np.float32),
         np.asarray(b_tail, np.float32)], axis=0)       # [544]
    perm = _build_perm()
    Wp, bp = W_all[perm], b_all[perm]

    # fp8 DoubleRow weights: [128, 8 (k-subtile), 544], x16 pre-scale
    wtb = np.ascontiguousarray(
        (WSCALE * Wp[:, :HID].T).reshape(8, 128, MTOT).transpose(1, 0, 2)
    ).astype(FP8).reshape(128, 8 * MTOT)
    # one-hot table: W_emb @ emb.T + bias, x16 to match the weight scale
    tbl = (WSCALE * (np.asarray(ent_emb, np.float64) @ Wp[:, HID:].T.astype(np.float64)
                     + bp[None, :].astype(np.float64))).astype(BF16)  # [3, 544]

    trig, jt, tril = _host_tables()

    in_maps = []
    for b in range(B):
        xT = np.ascontiguousarray(
            hidden[b].T.reshape(8, 128, S).transpose(1, 0, 2)
        ).astype(FP8).reshape(128, 8 * S)
        oh = (entity_labels[b][None, :] == np.arange(3)[:, None]).astype(BF16)
        in_maps.append(dict(xT=xT, oh=oh, wtb=wtb, tbl=tbl,
                            trig=trig, jt=jt, tril=tril))

    nc = _get_nc()
    res = run_bass_kernel_spmd(nc, in_maps, core_ids=list(range(NCORES)),
                               trace=_want_trace)

    gts = {0: np.asarray(gt_entity), 2: np.asarray(gt_head),
           3: np.asarray(gt_tail)}
    total = 0.0
    for b in range(B):
        out = res.results[b]
        sums = out["sums"].astype(np.float64)      # [128, SUMS_COLS]
        qkv = out["qkout"].astype(np.float64)      # [8, 68, 1024]
        qkv[4:] /= WSCALE                          # head/tail groups carry x16
        for h, (gq, gk, is_tril) in enumerate(_HEADS):
            E = float(np.sum(sums[:, _ACC_OFF[h]:_ACC_OFF[h + 1]]))
            if h < 2:
                gt = gts[0][b, h]
            else:
                gt = gts[h][b, 0]
            total += _mcce_host(E, qkv[gq], qkv[gk], gt)

    if _want_trace:
        kernel._last_results = res
    return np.array(total, dtype=np.float32)


# revision 8
# speedup vs baseline: 1.4164x; 1.0014x over previous
"""Bass/Trainium2 kernel for nn_GPREDecoder (GlobalPointer relation-extraction loss).

Strategy: data-parallel over batch (B=8 -> 8 cores, 1 example per core).
Per example on-device:
  - projT = W_all @ x_aug.T via fp8-e4m3 DoubleRow matmuls (2 k-tiles per
    PE pass; weights pre-scaled x16 for fp8 range) plus a small bf16
    one-hot matmul for the label-embedding + bias contribution
    (table = W_emb @ ent_emb.T + b precomputed exactly on host).
  - RoPE rotation for the two "ent" heads (J-matmul + cos/sin elementwise,
    trig tables pre-divided by 16 so the ent q/k come out exactly scaled)
  - per-head S x S logits tiles on PE (bf16), exp(scale*logit) on ACT with
    fused per-row accumulation -> per-head sum(exp(masked logits)); the
    S x S tensors never touch HBM.
  - outputs per-head exp-sums and the final q/k tensors (bf16).
The host gathers the 64 ground-truth pairs per head from q/k, applies the
multilabel-CE pos/neg log corrections in float64, and returns the scalar loss.
"""

import ml_dtypes
import numpy as np
from contextlib import ExitStack

import concourse.bass as bass
import concourse.mybir as mybir
import concourse.tile as tile
from concourse import bacc
from concourse.bass_utils import run_bass_kernel_spmd

B, S, HID, LAB = 8, 1024, 1024, 64
HD = 68
SCALE = 1.0 / HD**0.5
INF = 1.0e12
NCORES = 8
MTOT = 544   # total projection output channels
WSCALE = 16.0  # fp8 weight pre-scale (keeps 0.03-sigma weights in e4m3 normals)
NEG_BIG = -1.0e9  # additive pre-scale mask; exp(SCALE*NEG_BIG) == 0 in fp32

FP8 = ml_dtypes.float8_e4m3
BF16 = ml_dtypes.bfloat16

# group order: q_ent0 k_ent0 q_ent1 k_ent1 q_head k_head q_tail k_tail
_GROUP_ORIG = [0, 68, 136, 204, 272, 340, 408, 476]
# heads: (q_group, k_group, tril?)  heads 0,1 = ent (rope, exact scale);
# heads 2,3 = head/tail (q/k carry the x16 factor -> exp scale /256)
_HEADS = [(0, 1, True), (2, 3, True), (4, 5, False), (6, 7, False)]


def _spill_slots():
    """Destination (tile, row) slots for the 4 spill groups, in order."""
    slots = []
    for t in range(4):
        slots.extend((t, r) for r in range(68, 128))
    slots.extend((4, r) for r in range(32))
    return slots


def _build_perm():
    """perm[c_new] = original channel index, for the projection output layout."""
    perm = np.zeros(MTOT, np.int64)
    for g in range(4):  # rope groups aligned at row 0 of tiles 0..3
        perm[g * 128: g * 128 + 68] = np.arange(_GROUP_ORIG[g], _GROUP_ORIG[g] + 68)
    slots = _spill_slots()
    pos = 0
    for g in range(4, 8):
        for j in range(68):
            t, r = slots[pos]
            perm[t * 128 + r] = _GROUP_ORIG[g] + j
            pos += 1
    return perm


def _spill_pieces():
    """Per spill group: contiguous (src_tile, src_row0, cnt, dst_row0) DMA pieces."""
    slots = _spill_slots()
    out = {g: [] for g in range(4, 8)}
    pos = 0
    for g in range(4, 8):
        j = 0
        while j < 68:
            t, r = slots[pos]
            cnt = 1
            while j + cnt < 68 and pos + cnt < len(slots) and \
                    slots[pos + cnt] == (t, r + cnt):
                cnt += 1
            out[g].append((t, r, cnt, j))
            pos += cnt
            j += cnt
    return out


def _round_chunks(mtiles):
    """Chunk m-tiles of one [128,1024] psum round into bank-fitting matmul chunks.

    mtiles: [(m, local_start, width)] with local starts such that every
    <=512 chunk stays inside one 512-col bank. Returns
    [(m, local_off, src_off, n)].
    """
    chunks = []
    for (m, lo, w) in mtiles:
        off = 0
        while off < w:
            n = min(512 - ((lo + off) % 512), w - off)
            chunks.append((m, lo + off, off, n))
            off += n
    return chunks


def _head_rounds(is_tril, wide):
    """Per head: list of rounds; each round = (mtiles, span_end).

    Narrow rounds target [128, 1024] (2-bank) psum tiles, wide rounds
    [128, 2048] (4-bank). For tril heads the m-tile widths shrink (only
    columns >= 128*m are live), so m-tiles pack; spans stay contiguous.
    """
    if not is_tril:
        if not wide:
            return [([(m, 0, 1024)], 1024) for m in range(8)]
        return [([(2 * r, 0, 1024), (2 * r + 1, 1024, 1024)], 2048)
                for r in range(4)]
    widths = [1024 - 128 * m for m in range(8)]
    groups = ((0,), (1,), (2, 6), (3, 7), (4, 5)) if not wide else         ((0, 1), (2, 3, 6), (4, 5, 7))
    rounds = []
    for group in groups:
        mtiles = []
        local = 0
        for m in group:
            mtiles.append((m, local, widths[m]))
            local += widths[m]
        rounds.append((mtiles, local))
    return rounds


# all heads currently narrow (wide pool swap crashed the exec unit)
_HEAD_WIDE = [False, False, False, False]
_ACC_COLS = [len(_head_rounds(t, _HEAD_WIDE[h]))
             for h, (_, _, t) in enumerate(_HEADS)]         # [5, 3, 4, 4]
_ACC_OFF = np.concatenate([[0], np.cumsum(_ACC_COLS)])      # col offset per head
SUMS_COLS = int(_ACC_OFF[-1])                               # total accum columns


def _build_nc():
    f32 = mybir.dt.float32
    bf16 = mybir.dt.bfloat16
    fp8 = mybir.dt.float8e4
    Exp = mybir.ActivationFunctionType.Exp
    DR = mybir.MatmulPerfMode.DoubleRow

    nc = bacc.Bacc("TRN2", target_bir_lowering=False)

    xT = nc.dram_tensor("xT", [128, 8 * S], fp8, kind="ExternalInput")
    wtb = nc.dram_tensor("wtb", [128, 8 * MTOT], fp8, kind="ExternalInput")
    oh = nc.dram_tensor("oh", [3, S], bf16, kind="ExternalInput")
    tbl = nc.dram_tensor("tbl", [3, MTOT], bf16, kind="ExternalInput")
    trig = nc.dram_tensor("trig", [HD, 2 * S], bf16, kind="ExternalInput")
    jt = nc.dram_tensor("jt", [128, 128], bf16, kind="ExternalInput")
    tril = nc.dram_tensor("tril", [128, 128], f32, kind="ExternalInput")
    sums = nc.dram_tensor("sums", [128, SUMS_COLS], f32, kind="ExternalOutput")
    qkout = nc.dram_tensor("qkout", [8, HD, S], bf16, kind="ExternalOutput")

    xT_r = xT.rearrange("p (o f) -> p o f", f=S)        # [128, 8, 1024]
    wtb_r = wtb.rearrange("p (o f) -> p o f", f=MTOT)   # [128, 8, 544]

    with tile.TileContext(nc) as tc, ExitStack() as ctx:
        singles = ctx.enter_context(tc.tile_pool(name="singles", bufs=1))
        scratch = ctx.enter_context(tc.tile_pool(name="scratch", bufs=2))

        xT_sb = singles.tile([128, 8, S], fp8, tag="xT_sb", name="xT_sb")
        wtb_sb = singles.tile([128, 8, MTOT], fp8, tag="wtb_sb", name="wtb_sb")
        oh_sb = singles.tile([3, S], bf16, tag="oh_sb", name="oh_sb")
        tbl_sb = singles.tile([3, MTOT], bf16, tag="tbl_sb", name="tbl_sb")
        trig_sb = singles.tile([HD, 2 * S], bf16, tag="trig_sb", name="trig_sb")
        jt_sb = singles.tile([128, 128], bf16, tag="jt_sb", name="jt_sb")
        tril_sb = singles.tile([128, 128], f32, tag="tril_sb", name="tril_sb")
        dense = [singles.tile([128, S], bf16, tag=f"dense{t}", name=f"dense{t}")
                 for t in range(5)]
        qk = [singles.tile([HD, S], bf16, tag=f"qk{g}", name=f"qk{g}")
              for g in range(8)]
        sums_sb = singles.tile([128, SUMS_COLS], f32, tag="sums_sb", name="sums_sb")
        dummy = singles.tile([1, 8], f32, tag="dummy", name="dummy")
        warm_src = singles.tile([128, 512], bf16, tag="warm_src", name="warm_src")

        cos_sb = trig_sb[:, 0:S]
        sin_sb = trig_sb[:, S:2 * S]

        # Early: zero accumulators; pre-warm the ACT exp table load.
        nc.vector.memset(sums_sb[:], 0.0)
        nc.vector.memset(dummy[:], 0.0)
        nc.vector.memset(warm_src[:], 0.0)
        nc.scalar.activation(dummy[:], dummy[:], Exp)

        # Input DMAs. Triggers only on sync/vector/gpsimd queues (scalar
        # runs the exp stream, tensor the matmul stream). First k-pair of
        # weights+activations first so the projection starts ASAP.
        for p in range(4):
            nc.sync.dma_start(out=wtb_sb[:, 2 * p:2 * p + 2],
                              in_=wtb_r[:, 2 * p:2 * p + 2])
            nc.gpsimd.dma_start(out=xT_sb[:, 2 * p:2 * p + 2],
                                in_=xT_r[:, 2 * p:2 * p + 2])
            if p == 0:
                nc.sync.dma_start(out=tbl_sb[:], in_=tbl[:, :])
                nc.gpsimd.dma_start(out=oh_sb[:], in_=oh[:, :])
        nc.gpsimd.dma_start(out=trig_sb[:], in_=trig[:, :])
        nc.sync.dma_start(out=jt_sb[:], in_=jt[:, :])
        nc.sync.dma_start(out=tril_sb[:], in_=tril[:, :])

        # PSUM: proj ring (2x 2-bank) + head0/J ring (2x 2-bank) = 8 banks.
        ps = tc.alloc_tile_pool(name="ps", bufs=2, space="PSUM")
        hj = tc.alloc_tile_pool(name="hj", bufs=2, space="PSUM")

        def proj_pair(p, t, pt):
            lo = t * 128
            hi = min(lo + 128, MTOT)
            for c in (0, 512):
                nc.tensor.matmul(
                    pt[0:hi - lo, c:c + 512],
                    wtb_sb[:, 2 * p:2 * p + 2, lo:hi],
                    xT_sb[:, 2 * p:2 * p + 2, c:c + 512],
                    start=(p == 0), stop=False, perf_mode=DR,
                )

        def proj_oh(t, pt):
            lo = t * 128
            hi = min(lo + 128, MTOT)
            for c in (0, 512):
                nc.tensor.matmul(
                    pt[0:hi - lo, c:c + 512],
                    tbl_sb[:, lo:hi],
                    oh_sb[:, c:c + 512],
                    start=False, stop=True,
                )

        # spill pieces grouped by source tile; qkout[g] once g completes
        by_tile = {t: [] for t in range(5)}
        for g, pieces in _spill_pieces().items():
            for (t, r0, cnt, d0) in pieces:
                by_tile[t].append((g, r0, cnt, d0))
        g_left = {g: len(p) for g, p in _spill_pieces().items()}
        spill_eng = [nc.sync, nc.gpsimd]

        def spill_for_tile(t):
            for i, (g, r0, cnt, d0) in enumerate(by_tile[t]):
                spill_eng[(t + i) % 2].dma_start(
                    out=qk[g][d0:d0 + cnt, :], in_=dense[t][r0:r0 + cnt, :])
                g_left[g] -= 1
                if g_left[g] == 0:
                    spill_eng[(t + i) % 2].dma_start(
                        out=qkout[g], in_=qk[g][:, :])

        def evac_dve(t, pt):
            hi = min(128, MTOT - t * 128)
            nc.vector.tensor_copy(out=dense[t][0:hi, :], in_=pt[0:hi, :])
            spill_for_tile(t)

        def jrot(g):
            pj = hj.tile([128, S], f32, tag="hj", name=f"jq{g}")
            for c in (0, 512):
                nc.tensor.matmul(pj[0:HD, c:c + 512], jt_sb[:, 0:HD],
                                 dense[g][:, c:c + 512], start=True, stop=True)
            return pj

        def rope(g, pj):
            # qk[g] = dense[g]*(cos/16) + (J @ dense[g])*(sin/16) == exact q_rot
            nc.gpsimd.tensor_tensor(qk[g][:, :], dense[g][0:HD, :], cos_sb,
                                    mybir.AluOpType.mult)
            rtmp = scratch.tile([HD, S], bf16, tag="rtmp", name=f"rtmp{g}")
            nc.vector.tensor_tensor(rtmp[:, :], pj[0:HD, :], sin_sb,
                                    mybir.AluOpType.mult)
            nc.vector.tensor_tensor(qk[g][:, :], qk[g][:, :], rtmp[:, :],
                                    mybir.AluOpType.add)

        def head_round(h, ri, pool, tag, width):
            gq, gk, is_tril = _HEADS[h]
            sc = SCALE if h < 2 else SCALE / (WSCALE * WSCALE)
            mtiles, span = _head_rounds(is_tril, wide=(width == 2048))[ri]
            pl = pool.tile([128, width], f32, tag=tag, name=f"l{h}_{ri}")
            for (m, lo, so, n) in _round_chunks(mtiles):
                g0 = 128 * m if is_tril else 0
                nc.tensor.matmul(
                    pl[:, lo:lo + n],
                    qk[gq][:, m * 128:(m + 1) * 128],
                    qk[gk][:, g0 + so:g0 + so + n],
                    start=True, stop=True,
                )
            if is_tril:
                for (m, lo, w) in mtiles:
                    nc.vector.tensor_tensor(
                        pl[:, lo:lo + 128], pl[:, lo:lo + 128],
                        tril_sb, mybir.AluOpType.add)
            acc = int(_ACC_OFF[h]) + ri
            nc.scalar.activation(
                pl[:, 0:span], pl[:, 0:span], Exp, scale=sc,
                accum_out=sums_sb[:, acc:acc + 1])

        # ---- PE warm-up: keep the tensor engine streaming while the input
        # DMAs land so the p-state ramp reaches full clock before real work.
        wt = hj.tile([128, S], f32, tag="hj", name="warm")
        for _ in range(13):
            nc.tensor.matmul(wt[:, 0:512], warm_src[:, 0:128],
                             warm_src[:, 0:512], start=True, stop=True)

        # ---- proj tiles 0,1 (ent-h0 q/k); evac via ACT (idle till head0) ----
        pt0 = ps.tile([128, S], f32, tag="ps", name="proj0")
        pt1 = ps.tile([128, S], f32, tag="ps", name="proj1")
        for p in range(4):
            proj_pair(p, 0, pt0)
            proj_pair(p, 1, pt1)
        proj_oh(0, pt0)
        proj_oh(1, pt1)
        # half-grained evac -> J -> rope pipeline to start head0 ASAP
        pj0 = hj.tile([128, S], f32, tag="hj", name="jq0")
        pj1 = hj.tile([128, S], f32, tag="hj", name="jq1")
        for g, pt, pj in ((0, pt0, pj0), (1, pt1, pj1)):
            for c in (0, 512):
                nc.scalar.copy(out=dense[g][:, c:c + 512], in_=pt[:, c:c + 512])
                nc.tensor.matmul(pj[0:HD, c:c + 512], jt_sb[:, 0:HD],
                                 dense[g][:, c:c + 512], start=True, stop=True)
                nc.gpsimd.tensor_tensor(qk[g][:, c:c + 512],
                                        dense[g][0:HD, c:c + 512],
                                        cos_sb[:, c:c + 512],
                                        mybir.AluOpType.mult)
                rtmp = scratch.tile([HD, 512], bf16, tag="rtmp2",
                                    name=f"rt{g}_{c}")
                nc.vector.tensor_tensor(rtmp[:, :], pj[0:HD, c:c + 512],
                                        sin_sb[:, c:c + 512],
                                        mybir.AluOpType.mult)
                nc.vector.tensor_tensor(qk[g][:, c:c + 512],
                                        qk[g][:, c:c + 512], rtmp[:, :],
                                        mybir.AluOpType.add)
            spill_for_tile(g)
        nc.sync.dma_start(out=qkout[0], in_=qk[0][:, :])
        nc.gpsimd.dma_start(out=qkout[1], in_=qk[1][:, :])

        # ---- head0 rounds interleaved with proj tiles 2,3,4 on the PE ----
        pt2 = ps.tile([128, S], f32, tag="ps", name="proj2")
        proj_pair(0, 2, pt2)
        proj_pair(1, 2, pt2)
        head_round(0, 0, hj, "hj", 1024)
        proj_pair(2, 2, pt2)
        head_round(0, 1, hj, "hj", 1024)
        proj_pair(3, 2, pt2)
        proj_oh(2, pt2)
        head_round(0, 2, hj, "hj", 1024)
        evac_dve(2, pt2)
        pt3 = ps.tile([128, S], f32, tag="ps", name="proj3")
        proj_pair(0, 3, pt3)
        proj_pair(1, 3, pt3)
        head_round(0, 3, hj, "hj", 1024)
        proj_pair(2, 3, pt3)
        proj_pair(3, 3, pt3)
        proj_oh(3, pt3)
        evac_dve(3, pt3)
        pt4 = ps.tile([128, S], f32, tag="ps", name="proj4")
        proj_pair(0, 4, pt4)
        proj_pair(1, 4, pt4)
        head_round(0, 4, hj, "hj", 1024)
        proj_pair(2, 4, pt4)
        proj_pair(3, 4, pt4)
        proj_oh(4, pt4)
        evac_dve(4, pt4)

        # ---- rope for ent head 1 while the head/tail spill DMAs run ----
        pj2 = jrot(2)
        pj3 = jrot(3)
        rope(2, pj2)
        rope(3, pj3)
        nc.sync.dma_start(out=qkout[2], in_=qk[2][:, :])
        nc.gpsimd.dma_start(out=qkout[3], in_=qk[3][:, :])

        # ---- tail heads: narrow rounds alternating the two psum rings ----
        def sums_piece(h):
            a, b = int(_ACC_OFF[h]), int(_ACC_OFF[h + 1])
            nc.sync.dma_start(out=sums[:, a:b], in_=sums_sb[:, a:b])

        sums_piece(0)
        for hh, nr in ((2, 8), (3, 8), (1, 5)):
            for ri in range(nr):
                pool, tag = ((ps, "ps"), (hj, "hj"))[ri % 2]
                head_round(hh, ri, pool, tag, 1024)
            sums_piece(hh)
        hj.release()
        ps.release()

    nc.finalize()
    return nc


_NC_CACHE = None


def _get_nc():
    global _NC_CACHE
    if _NC_CACHE is None:
        _NC_CACHE = _build_nc()
    return _NC_CACHE


def _host_tables():
    pos = np.arange(S, dtype=np.float64)[:, None]
    inv = np.power(10000.0, -2.0 * np.arange(HD // 2, dtype=np.float64) / HD)
    ang = pos * inv                                   # [S, 34]
    trig = np.zeros((HD, 2 * S), np.float64)
    trig[:, 0:S] = np.repeat(np.cos(ang), 2, axis=1).T / WSCALE
    trig[:, S:2 * S] = np.repeat(np.sin(ang), 2, axis=1).T / WSCALE
    jt = np.zeros((128, 128), np.float32)
    for i in range(HD // 2):
        # J[2i, 2i+1] = -1 ; J[2i+1, 2i] = +1  -> stored transposed
        jt[2 * i + 1, 2 * i] = -1.0
        jt[2 * i, 2 * i + 1] = 1.0
    tril = np.where(np.arange(128)[None, :] >= np.arange(128)[:, None],
                    0.0, NEG_BIG).astype(np.float32)
    return trig.astype(BF16), jt.astype(BF16), tril


def _mcce_host(E_dev, q, k, gt):
    """pos/neg multilabel-CE for one (example, head). q,k: [68,S] f64; gt: [P,2]."""
    i = gt[:, 0].astype(np.int64)
    j = gt[:, 1].astype(np.int64)
    flat = i * S + j
    lv = np.sum(q[:, i] * k[:, j], axis=0) * SCALE    # [P]
    live = flat != 0
    pos_loss = np.log1p(np.sum(np.exp(-lv[live])))
    l00 = float(np.sum(q[:, 0] * k[:, 0]) * SCALE)
    uf, ui = np.unique(flat, return_index=True)
    keep = uf != 0
    excl = np.exp(l00) + np.sum(np.exp(lv[ui[keep]]))
    neg_loss = np.log1p(E_dev - excl)
    return pos_loss + neg_loss


def _reference_numpy(hidden, entity_labels, attention_mask, gt_entity, gt_head,
                     gt_tail, ent_emb, W_ent, b_ent, W_head, b_head, W_tail,
                     b_tail):
    """Slow exact numpy fallback (used only if attention_mask is not all-ones)."""
    x = np.concatenate([hidden, ent_emb[entity_labels]], axis=-1)

    def rope(v):
        b, s, h, d = v.shape
        pos = np.arange(s, dtype=np.float32)[:, None]
        inv = np.power(10000.0, -2.0 * np.arange(d // 2, dtype=np.float32) / d)
        ang = pos * inv
        sin = np.repeat(np.sin(ang), 2, axis=-1)[None, :, None, :]
        cos = np.repeat(np.cos(ang), 2, axis=-1)[None, :, None, :]
        v2 = np.stack([-v[..., 1::2], v[..., ::2]], axis=-1).reshape(v.shape)
        return v * cos + v2 * sin

    def gp(x, W, b, mask, heads, use_rope, tril):
        bx, sx, _ = x.shape
        proj = (x @ W.T + b).reshape(bx, sx, heads, 2 * HD)
        qw, kw = proj[..., :HD], proj[..., HD:]
        if use_rope:
            qw, kw = rope(qw), rope(kw)
        logits = np.einsum('bmhd,bnhd->bhmn', qw, kw) * SCALE
        pad = mask[:, None, None, :]
        logits = logits * pad - (1.0 - pad) * INF
        if tril:
            logits = logits - np.tril(np.ones((sx, sx), np.float32), -1) * INF
        return logits

    def mcce(y_true, y_pred):
        bx, hx, sx, _ = y_pred.shape
        flat = y_true[..., 0].astype(np.int64) * sx + y_true[..., 1]
        yp = y_pred.reshape(bx, hx, sx * sx).astype(np.float64)
        total = 0.0
        for b in range(bx):
            for h in range(hx):
                f = flat[b, h]
                live = f != 0
                lv = yp[b, h][f]
                pos = np.log1p(np.sum(np.exp(-lv[live])))
                neg_terms = yp[b, h].copy()
                neg_terms[0] = -np.inf
                neg_terms[np.unique(f)] = -np.inf
                neg = np.log1p(np.sum(np.exp(neg_terms)))
                total += pos + neg
        return total

    loss = 0.0
    loss += mcce(gt_entity, gp(x, W_ent, b_ent, attention_mask, 2, True, True))
    loss += mcce(gt_head, gp(x, W_head, b_head, attention_mask, 1, False, False))
    loss += mcce(gt_tail, gp(x, W_tail, b_tail, attention_mask, 1, False, False))
    return np.array(loss, dtype=np.float32)


def kernel(hidden, entity_labels, attention_mask, gt_entity, gt_head, gt_tail,
           ent_emb, W_ent, b_ent, W_head, b_head, W_tail, b_tail,
           _want_trace=False):
    hidden = np.asarray(hidden, np.float32)
    entity_labels = np.asarray(entity_labels)
    attention_mask = np.asarray(attention_mask, np.float32)
    ent_emb = np.asarray(ent_emb, np.float32)

    if not np.all(attention_mask == 1.0):
        return _reference_numpy(
            hidden, entity_labels, attention_mask, np.asarray(gt_entity),
            np.asarray(gt_head), np.asarray(gt_tail), ent_emb,
            np.asarray(W_ent, np.float32), np.asarray(b_ent, np.float32),
            np.asarray(W_head, np.float32), np.asarray(b_head, np.float32),
            np.asarray(W_tail, np.float32), np.asarray(b_tail, np.float32))

    W_all = np.concatenate(
        [np.asarray(W_ent, np.float32), np.asarray(W_head, np.float32),
         np.asarray(W_tail, np.float32)], axis=0)       # [544, 1088]
    b_all = np.concatenate(
        [np.asarray(b_ent, np.float32), np.asarray(b_head, np.float32),
         np.asarray(b_tail, np.float32)], axis=0)       # [544]
    perm = _build_perm()
    Wp, bp = W_all[perm], b_all[perm]

    # fp8 DoubleRow weights: [128, 8 (k-subtile), 544], x16 pre-scale
    wtb = np.ascontiguousarray(
        (WSCALE * Wp[:, :HID].T).reshape(8, 128, MTOT).transpose(1, 0, 2)
    ).astype(FP8).reshape(128, 8 * MTOT)
    # one-hot table: W_emb @ emb.T + bias, x16 to match the weight scale
    tbl = (WSCALE * (np.asarray(ent_emb, np.float64) @ Wp[:, HID:].T.astype(np.float64)
                     + bp[None, :].astype(np.float64))).astype(BF16)  # [3, 544]

    trig, jt, tril = _host_tables()

    in_maps = []
    for b in range(B):
        xT = np.ascontiguousarray(
            hidden[b].T.reshape(8, 128, S).transpose(1, 0, 2)
        ).astype(FP8).reshape(128, 8 * S)
        oh = (entity_labels[b][None, :] == np.arange(3)[:, None]).astype(BF16)
        in_maps.append(dict(xT=xT, oh=oh, wtb=wtb, tbl=tbl,
                            trig=trig, jt=jt, tril=tril))

    nc = _get_nc()
    res = run_bass_kernel_spmd(nc, in_maps, core_ids=list(range(NCORES)),
                               trace=_want_trace)

    gts = {0: np.asarray(gt_entity), 2: np.asarray(gt_head),
           3: np.asarray(gt_tail)}
    total = 0.0
    for b in range(B):
        out = res.results[b]
        sums = out["sums"].astype(np.float64)      # [128, SUMS_COLS]
        qkv = out["qkout"].astype(np.float64)      # [8, 68, 1024]
        qkv[4:] /= WSCALE                          # head/tail groups carry x16
        for h, (gq, gk, is_tril) in enumerate(_HEADS):
            E = float(np.sum(sums[:, _ACC_OFF[h]:_ACC_OFF[h + 1]]))
            if h < 2:
                gt = gts[0][b, h]
            else:
                gt = gts[h][b, 0]
            total += _mcce_host(E, qkv[gq], qkv[gk], gt)

    if _want_trace:
        kernel._last_results = res
    return np.array(total, dtype=np.float32)


# revision 10
# speedup vs baseline: 1.4497x; 1.0235x over previous
"""Bass/Trainium2 kernel for nn_GPREDecoder (GlobalPointer relation-extraction loss).

Strategy: data-parallel over batch (B=8 -> 8 cores, 1 example per core).
Per example on-device:
  - projT = W_all @ x_aug.T via fp8-e4m3 DoubleRow matmuls (2 k-tiles per
    PE pass; weights pre-scaled x16 for fp8 range) plus a small bf16
    one-hot matmul for the label-embedding + bias contribution
    (table = W_emb @ ent_emb.T + b precomputed exactly on host).
  - RoPE rotation for the two "ent" heads (J-matmul + cos/sin elementwise,
    trig tables pre-divided by 16 so the ent q/k come out exactly scaled)
  - per-head S x S logits tiles on PE (bf16), exp(scale*logit) on ACT with
    fused per-row accumulation -> per-head sum(exp(masked logits)); the
    S x S tensors never touch HBM.
  - outputs per-head exp-sums and the final q/k tensors (bf16).
The host gathers the 64 ground-truth pairs per head from q/k, applies the
multilabel-CE pos/neg log corrections in float64, and returns the scalar loss.
"""

import ml_dtypes
import numpy as np
from contextlib import ExitStack

import concourse.bass as bass
import concourse.mybir as mybir
import concourse.tile as tile
from concourse import bacc
from concourse.bass_utils import run_bass_kernel_spmd

B, S, HID, LAB = 8, 1024, 1024, 64
HD = 68
SCALE = 1.0 / HD**0.5
INF = 1.0e12
NCORES = 8
MTOT = 544   # total projection output channels
WSCALE = 16.0  # fp8 weight pre-scale (keeps 0.03-sigma weights in e4m3 normals)
NEG_BIG = -1.0e9  # additive pre-scale mask; exp(SCALE*NEG_BIG) == 0 in fp32

FP8 = ml_dtypes.float8_e4m3
BF16 = ml_dtypes.bfloat16

# group order: q_ent0 k_ent0 q_ent1 k_ent1 q_head k_head q_tail k_tail
_GROUP_ORIG = [0, 68, 136, 204, 272, 340, 408, 476]
# heads: (q_group, k_group, tril?)  heads 0,1 = ent (rope, exact scale);
# heads 2,3 = head/tail (q/k carry the x16 factor -> exp scale /256)
_HEADS = [(0, 1, True), (2, 3, True), (4, 5, False), (6, 7, False)]


def _spill_slots():
    """Destination (tile, row) slots for the 4 spill groups, in order."""
    slots = []
    for t in range(4):
        slots.extend((t, r) for r in range(68, 128))
    slots.extend((4, r) for r in range(32))
    return slots


def _build_perm():
    """perm[c_new] = original channel index, for the projection output layout."""
    perm = np.zeros(MTOT, np.int64)
    for g in range(4):  # rope groups aligned at row 0 of tiles 0..3
        perm[g * 128: g * 128 + 68] = np.arange(_GROUP_ORIG[g], _GROUP_ORIG[g] + 68)
    slots = _spill_slots()
    pos = 0
    for g in range(4, 8):
        for j in range(68):
            t, r = slots[pos]
            perm[t * 128 + r] = _GROUP_ORIG[g] + j
            pos += 1
    return perm


def _spill_pieces():
    """Per spill group: contiguous (src_tile, src_row0, cnt, dst_row0) DMA pieces."""
    slots = _spill_slots()
    out = {g: [] for g in range(4, 8)}
    pos = 0
    for g in range(4, 8):
        j = 0
        while j < 68:
            t, r = slots[pos]
            cnt = 1
            while j + cnt < 68 and pos + cnt < len(slots) and \
                    slots[pos + cnt] == (t, r + cnt):
                cnt += 1
            out[g].append((t, r, cnt, j))
            pos += cnt
            j += cnt
    return out


def _round_chunks(mtiles):
    """Chunk m-tiles of one [128,1024] psum round into bank-fitting matmul chunks.

    mtiles: [(m, local_start, width)] with local starts such that every
    <=512 chunk stays inside one 512-col bank. Returns
    [(m, local_off, src_off, n)].
    """
    chunks = []
    for (m, lo, w) in mtiles:
        off = 0
        while off < w:
            n = min(512 - ((lo + off) % 512), w - off)
            chunks.append((m, lo + off, off, n))
            off += n
    return chunks


def _head_rounds(is_tril, wide):
    """Per head: list of rounds; each round = (mtiles, span_end).

    Narrow rounds target [128, 1024] (2-bank) psum tiles, wide rounds
    [128, 2048] (4-bank). For tril heads the m-tile widths shrink (only
    columns >= 128*m are live), so m-tiles pack; spans stay contiguous.
    """
    if not is_tril:
        if not wide:
            return [([(m, 0, 1024)], 1024) for m in range(8)]
        return [([(2 * r, 0, 1024), (2 * r + 1, 1024, 1024)], 2048)
                for r in range(4)]
    widths = [1024 - 128 * m for m in range(8)]
    groups = ((0,), (1,), (2, 6), (3, 7), (4, 5)) if not wide else         ((0, 1), (2, 3, 6), (4, 5, 7))
    rounds = []
    for group in groups:
        mtiles = []
        local = 0
        for m in group:
            mtiles.append((m, local, widths[m]))
            local += widths[m]
        rounds.append((mtiles, local))
    return rounds


# all heads currently narrow (wide pool swap crashed the exec unit)
_HEAD_WIDE = [False, False, False, False]
_ACC_COLS = [len(_head_rounds(t, _HEAD_WIDE[h]))
             for h, (_, _, t) in enumerate(_HEADS)]         # [5, 3, 4, 4]
_ACC_OFF = np.concatenate([[0], np.cumsum(_ACC_COLS)])      # col offset per head
SUMS_COLS = int(_ACC_OFF[-1])                               # total accum columns


def _build_nc():
    f32 = mybir.dt.float32
    bf16 = mybir.dt.bfloat16
    fp8 = mybir.dt.float8e4
    Exp = mybir.ActivationFunctionType.Exp
    DR = mybir.MatmulPerfMode.DoubleRow

    nc = bacc.Bacc("TRN2", target_bir_lowering=False)

    xT = nc.dram_tensor("xT", [128, 8 * S], fp8, kind="ExternalInput")
    wtbA = nc.dram_tensor("wtbA", [128, 8 * 256], fp8, kind="ExternalInput")
    wtbB = nc.dram_tensor("wtbB", [128, 8 * 288], fp8, kind="ExternalInput")
    oh = nc.dram_tensor("oh", [3, S], bf16, kind="ExternalInput")
    tbl = nc.dram_tensor("tbl", [3, MTOT], bf16, kind="ExternalInput")
    trig = nc.dram_tensor("trig", [HD, 2 * S], bf16, kind="ExternalInput")
    jt = nc.dram_tensor("jt", [128, 128], bf16, kind="ExternalInput")
    tril = nc.dram_tensor("tril", [128, 128], f32, kind="ExternalInput")
    sums = nc.dram_tensor("sums", [128, SUMS_COLS], f32, kind="ExternalOutput")
    qkout = nc.dram_tensor("qkout", [8, HD, S], bf16, kind="ExternalOutput")

    xT_r = xT.rearrange("p (o f) -> p o f", f=S)        # [128, 8, 1024]
    wtbA_r = wtbA.rearrange("p (o f) -> p o f", f=256)  # [128, 8, 256]
    wtbB_r = wtbB.rearrange("p (o f) -> p o f", f=288)  # [128, 8, 288]

    with tile.TileContext(nc) as tc, ExitStack() as ctx:
        singles = ctx.enter_context(tc.tile_pool(name="singles", bufs=1))
        scratch = ctx.enter_context(tc.tile_pool(name="scratch", bufs=2))

        xT_sb = singles.tile([128, 8, S], fp8, tag="xT_sb", name="xT_sb")
        wtbA_sb = singles.tile([128, 8, 256], fp8, tag="wtbA_sb", name="wtbA_sb")
        wtbB_sb = singles.tile([128, 8, 288], fp8, tag="wtbB_sb", name="wtbB_sb")
        oh_sb = singles.tile([3, S], bf16, tag="oh_sb", name="oh_sb")
        tbl_sb = singles.tile([3, MTOT], bf16, tag="tbl_sb", name="tbl_sb")
        trig_sb = singles.tile([HD, 2 * S], bf16, tag="trig_sb", name="trig_sb")
        jt_sb = singles.tile([128, 128], bf16, tag="jt_sb", name="jt_sb")
        tril_sb = singles.tile([128, 128], f32, tag="tril_sb", name="tril_sb")
        dense = [singles.tile([128, S], bf16, tag=f"dense{t}", name=f"dense{t}")
                 for t in range(5)]
        qk = [singles.tile([HD, S], bf16, tag=f"qk{g}", name=f"qk{g}")
              for g in range(8)]
        sums_sb = singles.tile([128, SUMS_COLS], f32, tag="sums_sb", name="sums_sb")
        dummy = singles.tile([1, 8], f32, tag="dummy", name="dummy")
        warm_src = singles.tile([128, 512], bf16, tag="warm_src", name="warm_src")

        cos_sb = trig_sb[:, 0:S]
        sin_sb = trig_sb[:, S:2 * S]

        # Early: zero accumulators; pre-warm the ACT exp table load.
        nc.vector.memset(sums_sb[:], 0.0)
        nc.vector.memset(dummy[:], 0.0)
        nc.vector.memset(warm_src[:], 0.0)
        nc.scalar.activation(dummy[:], dummy[:], Exp)

        # Input DMAs, spread over the three DMA-capable queues (scalar is
        # free until the first evac at ~15us). Critical order: the four
        # xT k-pairs + the tiles-0/1 weight slices feed proj01; everything
        # else (wtbB, tables) arrives behind them.
        def wA(p, eng):
            eng.dma_start(out=wtbA_sb[:, 2 * p:2 * p + 2],
                          in_=wtbA_r[:, 2 * p:2 * p + 2])

        def wB(p, eng):
            eng.dma_start(out=wtbB_sb[:, 2 * p:2 * p + 2],
                          in_=wtbB_r[:, 2 * p:2 * p + 2])

        def xP(p, eng):
            eng.dma_start(out=xT_sb[:, 2 * p:2 * p + 2],
                          in_=xT_r[:, 2 * p:2 * p + 2])

        xP(0, nc.gpsimd)
        wA(0, nc.sync)
        xP(1, nc.scalar)
        wA(1, nc.sync)
        xP(2, nc.gpsimd)
        wA(2, nc.sync)
        xP(3, nc.scalar)
        wA(3, nc.sync)
        nc.sync.dma_start(out=oh_sb[:], in_=oh[:, :])
        nc.sync.dma_start(out=tbl_sb[:], in_=tbl[:, :])
        nc.gpsimd.dma_start(out=jt_sb[:], in_=jt[:, :])
        nc.gpsimd.dma_start(out=trig_sb[:], in_=trig[:, :])
        nc.gpsimd.dma_start(out=tril_sb[:], in_=tril[:, :])
        for p in range(4):
            wB(p, nc.sync)

        # PSUM: proj ring (2x 2-bank) + head0/J ring (2x 2-bank) = 8 banks.
        ps = tc.alloc_tile_pool(name="ps", bufs=2, space="PSUM")
        hj = tc.alloc_tile_pool(name="hj", bufs=2, space="PSUM")

        def proj_pair(p, t, pt):
            wsb = wtbA_sb if t < 2 else wtbB_sb
            lo = t * 128 if t < 2 else (t - 2) * 128
            hi = lo + (128 if t < 4 else 32)
            for c in (0, 512):
                nc.tensor.matmul(
                    pt[0:hi - lo, c:c + 512],
                    wsb[:, 2 * p:2 * p + 2, lo:hi],
                    xT_sb[:, 2 * p:2 * p + 2, c:c + 512],
                    start=(p == 0), stop=False, perf_mode=DR,
                )

        def proj_oh(t, pt):
            lo = t * 128
            hi = min(lo + 128, MTOT)  # tbl stays [3, 544] full-width
            for c in (0, 512):
                nc.tensor.matmul(
                    pt[0:hi - lo, c:c + 512],
                    tbl_sb[:, lo:hi],
                    oh_sb[:, c:c + 512],
                    start=False, stop=True,
                )

        # spill pieces grouped by source tile; qkout[g] once g completes
        by_tile = {t: [] for t in range(5)}
        for g, pieces in _spill_pieces().items():
            for (t, r0, cnt, d0) in pieces:
                by_tile[t].append((g, r0, cnt, d0))
        g_left = {g: len(p) for g, p in _spill_pieces().items()}
        spill_eng = [nc.sync, nc.gpsimd]

        def spill_for_tile(t):
            for i, (g, r0, cnt, d0) in enumerate(by_tile[t]):
                spill_eng[(t + i) % 2].dma_start(
                    out=qk[g][d0:d0 + cnt, :], in_=dense[t][r0:r0 + cnt, :])
                g_left[g] -= 1
                if g_left[g] == 0:
                    spill_eng[(t + i) % 2].dma_start(
                        out=qkout[g], in_=qk[g][:, :])

        def evac_dve(t, pt):
            hi = min(128, MTOT - t * 128)
            nc.vector.tensor_copy(out=dense[t][0:hi, :], in_=pt[0:hi, :])
            spill_for_tile(t)

        def jrot(g):
            pj = hj.tile([128, S], f32, tag="hj", name=f"jq{g}")
            for c in (0, 512):
                nc.tensor.matmul(pj[0:HD, c:c + 512], jt_sb[:, 0:HD],
                                 dense[g][:, c:c + 512], start=True, stop=True)
            return pj

        def rope(g, pj):
            # qk[g] = dense[g]*(cos/16) + (J @ dense[g])*(sin/16) == exact q_rot
            ta = scratch.tile([HD, S], bf16, tag="rtA", name=f"rtA{g}")
            tb = scratch.tile([HD, S], bf16, tag="rtB", name=f"rtB{g}")
            nc.vector.tensor_tensor(ta[:, :], dense[g][0:HD, :], cos_sb,
                                    mybir.AluOpType.mult)
            nc.vector.tensor_tensor(tb[:, :], pj[0:HD, :], sin_sb,
                                    mybir.AluOpType.mult)
            nc.vector.tensor_tensor(qk[g][:, :], ta[:, :], tb[:, :],
                                    mybir.AluOpType.add)

        def head_round(h, ri, pool, tag, width):
            gq, gk, is_tril = _HEADS[h]
            sc = SCALE if h < 2 else SCALE / (WSCALE * WSCALE)
            mtiles, span = _head_rounds(is_tril, wide=(width == 2048))[ri]
            pl = pool.tile([128, width], f32, tag=tag, name=f"l{h}_{ri}")
            for (m, lo, so, n) in _round_chunks(mtiles):
                g0 = 128 * m if is_tril else 0
                nc.tensor.matmul(
                    pl[:, lo:lo + n],
                    qk[gq][:, m * 128:(m + 1) * 128],
                    qk[gk][:, g0 + so:g0 + so + n],
                    start=True, stop=True,
                )
            if is_tril:
                for (m, lo, w) in mtiles:
                    nc.vector.tensor_tensor(
                        pl[:, lo:lo + 128], pl[:, lo:lo + 128],
                        tril_sb, mybir.AluOpType.add)
            acc = int(_ACC_OFF[h]) + ri
            nc.scalar.activation(
                pl[:, 0:span], pl[:, 0:span], Exp, scale=sc,
                accum_out=sums_sb[:, acc:acc + 1])

        # ---- PE warm-up: keep the tensor engine streaming while the input
        # DMAs land so the p-state ramp reaches full clock before real work.
        wt = hj.tile([128, S], f32, tag="hj", name="warm")
        for _ in range(7):
            nc.tensor.matmul(wt[:, 0:512], warm_src[:, 0:128],
                             warm_src[:, 0:512], start=True, stop=True)

        # ---- proj tiles 0,1 (ent-h0 q/k); evac via ACT (idle till head0) ----
        pt0 = ps.tile([128, S], f32, tag="ps", name="proj0")
        pt1 = ps.tile([128, S], f32, tag="ps", name="proj1")
        for p in range(4):
            proj_pair(p, 0, pt0)
            proj_pair(p, 1, pt1)
        proj_oh(0, pt0)
        proj_oh(1, pt1)
        nc.scalar.copy(out=dense[0][:, :], in_=pt0[:, :])
        nc.scalar.copy(out=dense[1][:, :], in_=pt1[:, :])
        pj0 = jrot(0)
        pj1 = jrot(1)
        rope(0, pj0)
        rope(1, pj1)
        spill_for_tile(0)
        spill_for_tile(1)
        nc.sync.dma_start(out=qkout[0], in_=qk[0][:, :])
        nc.gpsimd.dma_start(out=qkout[1], in_=qk[1][:, :])

        # ---- head0 ASAP; proj tiles 2,3,4 fill the PE between rounds ----
        head_round(0, 0, hj, "hj", 1024)
        pt2 = ps.tile([128, S], f32, tag="ps", name="proj2")
        proj_pair(0, 2, pt2)
        proj_pair(1, 2, pt2)
        head_round(0, 1, hj, "hj", 1024)
        proj_pair(2, 2, pt2)
        proj_pair(3, 2, pt2)
        proj_oh(2, pt2)
        head_round(0, 2, hj, "hj", 1024)
        evac_dve(2, pt2)
        pt3 = ps.tile([128, S], f32, tag="ps", name="proj3")
        proj_pair(0, 3, pt3)
        proj_pair(1, 3, pt3)
        head_round(0, 3, hj, "hj", 1024)
        proj_pair(2, 3, pt3)
        proj_pair(3, 3, pt3)
        proj_oh(3, pt3)
        evac_dve(3, pt3)
        pt4 = ps.tile([128, S], f32, tag="ps", name="proj4")
        proj_pair(0, 4, pt4)
        proj_pair(1, 4, pt4)
        head_round(0, 4, hj, "hj", 1024)
        proj_pair(2, 4, pt4)
        proj_pair(3, 4, pt4)
        proj_oh(4, pt4)
        evac_dve(4, pt4)

        # ---- rope for ent head 1 while the head/tail spill DMAs run ----
        pj2 = jrot(2)
        pj3 = jrot(3)
        rope(2, pj2)
        rope(3, pj3)
        nc.sync.dma_start(out=qkout[2], in_=qk[2][:, :])
        nc.gpsimd.dma_start(out=qkout[3], in_=qk[3][:, :])

        # ---- tail heads: narrow rounds alternating the two psum rings ----
        def sums_piece(h):
            a, b = int(_ACC_OFF[h]), int(_ACC_OFF[h + 1])
            nc.sync.dma_start(out=sums[:, a:b], in_=sums_sb[:, a:b])

        sums_piece(0)
        for hh, nr in ((2, 8), (3, 8), (1, 5)):
            for ri in range(nr):
                pool, tag = ((ps, "ps"), (hj, "hj"))[ri % 2]
                head_round(hh, ri, pool, tag, 1024)
            sums_piece(hh)
        hj.release()
        ps.release()

    nc.finalize()
    return nc


_NC_CACHE = None


def _get_nc():
    global _NC_CACHE
    if _NC_CACHE is None:
        _NC_CACHE = _build_nc()
    return _NC_CACHE


def _host_tables():
    pos = np.arange(S, dtype=np.float64)[:, None]
    inv = np.power(10000.0, -2.0 * np.arange(HD // 2, dtype=np.float64) / HD)
    ang = pos * inv                                   # [S, 34]
    trig = np.zeros((HD, 2 * S), np.float64)
    trig[:, 0:S] = np.repeat(np.cos(ang), 2, axis=1).T / WSCALE
    trig[:, S:2 * S] = np.repeat(np.sin(ang), 2, axis=1).T / WSCALE
    jt = np.zeros((128, 128), np.float32)
    for i in range(HD // 2):
        # J[2i, 2i+1] = -1 ; J[2i+1, 2i] = +1  -> stored transposed
        jt[2 * i + 1, 2 * i] = -1.0
        jt[2 * i, 2 * i + 1] = 1.0
    tril = np.where(np.arange(128)[None, :] >= np.arange(128)[:, None],
                    0.0, NEG_BIG).astype(np.float32)
    return trig.astype(BF16), jt.astype(BF16), tril


def _mcce_host(E_dev, q, k, gt):
    """pos/neg multilabel-CE for one (example, head). q,k: [68,S] f64; gt: [P,2]."""
    i = gt[:, 0].astype(np.int64)
    j = gt[:, 1].astype(np.int64)
    flat = i * S + j
    lv = np.sum(q[:, i] * k[:, j], axis=0) * SCALE    # [P]
    live = flat != 0
    pos_loss = np.log1p(np.sum(np.exp(-lv[live])))
    l00 = float(np.sum(q[:, 0] * k[:, 0]) * SCALE)
    uf, ui = np.unique(flat, return_index=True)
    keep = uf != 0
    excl = np.exp(l00) + np.sum(np.exp(lv[ui[keep]]))
    neg_loss = np.log1p(E_dev - excl)
    return pos_loss + neg_loss


def _reference_numpy(hidden, entity_labels, attention_mask, gt_entity, gt_head,
                     gt_tail, ent_emb, W_ent, b_ent, W_head, b_head, W_tail,
                     b_tail):
    """Slow exact numpy fallback (used only if attention_mask is not all-ones)."""
    x = np.concatenate([hidden, ent_emb[entity_labels]], axis=-1)

    def rope(v):
        b, s, h, d = v.shape
        pos = np.arange(s, dtype=np.float32)[:, None]
        inv = np.power(10000.0, -2.0 * np.arange(d // 2, dtype=np.float32) / d)
        ang = pos * inv
        sin = np.repeat(np.sin(ang), 2, axis=-1)[None, :, None, :]
        cos = np.repeat(np.cos(ang), 2, axis=-1)[None, :, None, :]
        v2 = np.stack([-v[..., 1::2], v[..., ::2]], axis=-1).reshape(v.shape)
        return v * cos + v2 * sin

    def gp(x, W, b, mask, heads, use_rope, tril):
        bx, sx, _ = x.shape
        proj = (x @ W.T + b).reshape(bx, sx, heads, 2 * HD)
        qw, kw = proj[..., :HD], proj[..., HD:]
        if use_rope:
            qw, kw = rope(qw), rope(kw)
        logits = np.einsum('bmhd,bnhd->bhmn', qw, kw) * SCALE
        pad = mask[:, None, None, :]
        logits = logits * pad - (1.0 - pad) * INF
        if tril:
            logits = logits - np.tril(np.ones((sx, sx), np.float32), -1) * INF
        return logits

    def mcce(y_true, y_pred):
        bx, hx, sx, _ = y_pred.shape
        flat = y_true[..., 0].astype(np.int64) * sx + y_true[..., 1]
        yp = y_pred.reshape(bx, hx, sx * sx).astype(np.float64)
        total = 0.0
        for b in range(bx):
            for h in range(hx):
                f = flat[b, h]
                live = f != 0
                lv = yp[b, h][f]
                pos = np.log1p(np.sum(np.exp(-lv[live])))
                neg_terms = yp[b, h].copy()
                neg_terms[0] = -np.inf
                neg_terms[np.unique(f)] = -np.inf
                neg = np.log1p(np.sum(np.exp(neg_terms)))
                total += pos + neg
        return total

    loss = 0.0
    loss += mcce(gt_entity, gp(x, W_ent, b_ent, attention_mask, 2, True, True))
    loss += mcce(gt_head, gp(x, W_head, b_head, attention_mask, 1, False, False))
    loss += mcce(gt_tail, gp(x, W_tail, b_tail, attention_mask, 1, False, False))
    return np.array(loss, dtype=np.float32)


def kernel(hidden, entity_labels, attention_mask, gt_entity, gt_head, gt_tail,
           ent_emb, W_ent, b_ent, W_head, b_head, W_tail, b_tail,
           _want_trace=False):
    hidden = np.asarray(hidden, np.float32)
    entity_labels = np.asarray(entity_labels)
    attention_mask = np.asarray(attention_mask, np.float32)
    ent_emb = np.asarray(ent_emb, np.float32)

    if not np.all(attention_mask == 1.0):
        return _reference_numpy(
            hidden, entity_labels, attention_mask, np.asarray(gt_entity),
            np.asarray(gt_head), np.asarray(gt_tail), ent_emb,
            np.asarray(W_ent, np.float32), np.asarray(b_ent, np.float32),
            np.asarray(W_head, np.float32), np.asarray(b_head, np.float32),
            np.asarray(W_tail, np.float32), np.asarray(b_tail, np.float32))

    W_all = np.concatenate(
        [np.asarray(W_ent, np.float32), np.asarray(W_head, np.float32),
         np.asarray(W_tail, np.float32)], axis=0)       # [544, 1088]
    b_all = np.concatenate(
        [np.asarray(b_ent, np.float32), np.asarray(b_head, np.float32),
         np.asarray(b_tail, np.float32)], axis=0)       # [544]
    perm = _build_perm()
    Wp, bp = W_all[perm], b_all[perm]

    # fp8 DoubleRow weights: [128, 8 (k-subtile), 544], x16 pre-scale,
    # split into tiles 0-1 (feeds proj01 first) and tiles 2-4
    wtb = np.ascontiguousarray(
        (WSCALE * Wp[:, :HID].T).reshape(8, 128, MTOT).transpose(1, 0, 2)
    ).astype(FP8)
    wtbA = np.ascontiguousarray(wtb[:, :, 0:256]).reshape(128, 8 * 256)
    wtbB = np.ascontiguousarray(wtb[:, :, 256:544]).reshape(128, 8 * 288)
    # one-hot table: W_emb @ emb.T + bias, x16 to match the weight scale
    tbl = (WSCALE * (np.asarray(ent_emb, np.float64) @ Wp[:, HID:].T.astype(np.float64)
                     + bp[None, :].astype(np.float64))).astype(BF16)  # [3, 544]

    trig, jt, tril = _host_tables()

    in_maps = []
    for b in range(B):
        xT = np.ascontiguousarray(
            hidden[b].T.reshape(8, 128, S).transpose(1, 0, 2)
        ).astype(FP8).reshape(128, 8 * S)
        oh = (entity_labels[b][None, :] == np.arange(3)[:, None]).astype(BF16)
        in_maps.append(dict(xT=xT, oh=oh, wtbA=wtbA, wtbB=wtbB, tbl=tbl,
                            trig=trig, jt=jt, tril=tril))

    nc = _get_nc()
    res = run_bass_kernel_spmd(nc, in_maps, core_ids=list(range(NCORES)),
                               trace=_want_trace)

    gts = {0: np.asarray(gt_entity), 2: np.asarray(gt_head),
           3: np.asarray(gt_tail)}
    total = 0.0
    for b in range(B):
        out = res.results[b]
        sums = out["sums"].astype(np.float64)      # [128, SUMS_COLS]
        qkv = out["qkout"].astype(np.float64)      # [8, 68, 1024]
        qkv[4:] /= WSCALE                          # head/tail groups carry x16
        for h, (gq, gk, is_tril) in enumerate(_HEADS):
            E = float(np.sum(sums[:, _ACC_OFF[h]:_ACC_OFF[h + 1]]))
            if h < 2:
                gt = gts[0][b, h]
            else:
                gt = gts[h][b, 0]
            total += _mcce_host(E, qkv[gq], qkv[gk], gt)

    if _want_trace:
        kernel._last_results = res
    return np.array(total, dtype=np.float32)
